# revision 27
# baseline (speedup 1.0000x reference)
"""HGRNBitMLP (BitNet-style SwiGLU MLP) on 8 TRN2 NeuronCores.

Data-parallel over the 4096 tokens (512/core). Weight ternarization is
sharded 1/8 per core; ternary weights are stored as fp8e4 (exact for
{-1,0,+1}) and AllGathered in fp8 — half the bytes of bf16. The global
mean(|w|) rides ONE tiny AllGather (lower floor than AllReduce) with an
on-PE partition sum. Activations are quantized to the int8 grid (exact
in bf16), so every matmul is an exact-integer fp8xbf16 matmul with f32
PSUM accumulation. Per-token scales are applied outside the matmuls.

Engine-queue discipline (strict FIFO per engine):
 - x reads are front-loaded on the sync ring; x-quant issues first on
   the DVE queue with its big elementwise ops offloaded to ACT.
 - 16 gate chunks map 1:1 onto mm1's sb blocks; chunks 2-15 and the
   down-proj tern are EMITTED INSIDE the mm1 loop (tern DVE interleaves
   with SwiGLU; DMA rides the gpsimd SWDGE ring).
 - the down-proj AllGather is split into two H-half AGs (A for mm2's
   first half, B for the second) so no single 16.8MB burst
   oversubscribes HBM against mm1's streaming.

Layouts: x is loaded [tok, h], quantized, PE-transposed to xqT [h, tok].
mm1 produces y^T tiles [o, tok]; SwiGLU keeps h as [I, tok] (f32,
spilled to DRAM) so mm2's operand q2T [I, tok] needs no transpose.
mm2 is j-outer over I with quantization fused into the first H-half
(q2T cached in SBUF bf16 for the second half).
"""
import sys

try:
    import concourse  # noqa: F401
except ImportError:
    sys.path.insert(0, "/opt/trn_rl_repo")

import numpy as np

import concourse.tile as tile
from concourse import bacc, mybir
from concourse.bass_utils import run_bass_kernel_spmd
from concourse.masks import make_identity

F32, BF16, F8 = mybir.dt.float32, mybir.dt.bfloat16, mybir.dt.float8e4
Alu = mybir.AluOpType
Act = mybir.ActivationFunctionType
X = mybir.AxisListType.X

NC_N = 8
B, S, H, I = 2, 2048, 2048, 8192
O2 = 2 * I
TOK = B * S
TPC = TOK // NC_N   # 512 tokens/core
TT = TPC // 128     # 4 token tiles
HK = H // 128       # 16 h tiles
IK = I // 128       # 64 I tiles
GSH = H // NC_N     # 256 rows of w_gate^T per core
DSH = I // NC_N     # 1024 rows of w_down^T per core
EPS = 1e-5
C_MAGIC = 12582912.0  # 1.5*2^23; (x+C)-C rounds f32 to nearest-even int
CCHUNK = 2048
NCH = 16            # gate AG chunks; chunk k pairs 512 gate + 512 v cols
GW = O2 // NCH // 2  # 512 och of gate (and of v) per chunk


def build(nc):
    x_ap = nc.dram_tensor("x", [TPC, H], F32, kind="ExternalInput").ap()
    wg_ap = nc.dram_tensor("wgt", [GSH, O2], F32, kind="ExternalInput").ap()
    wd_ap = nc.dram_tensor("wdt", [DSH, H], F32, kind="ExternalInput").ap()
    gg_ap = nc.dram_tensor("gg", [1, H], F32, kind="ExternalInput").ap()
    gd_ap = nc.dram_tensor("gdc", [128, IK], F32, kind="ExternalInput").ap()
    y_ap = nc.dram_tensor("y", [TPC, H], F32, kind="ExternalOutput").ap()
    rg = [list(range(NC_N))]

    with tile.TileContext(nc) as tc:
        with tc.tile_pool(name="dram", bufs=1, space="DRAM") as dram, \
             tc.tile_pool(name="perm", bufs=1) as cp, \
             tc.tile_pool(name="colp", bufs=1) as colp:

            # ---- front-load the input reads on the sync ring ----
            xt_p = tc.tile_pool(name="xtp", bufs=1)
            xt_pool = xt_p.__enter__()
            xts = []
            for t in range(TT):
                xt = xt_pool.tile([128, H], F32, name=f"xt{t}")
                nc.sync.dma_start(xt[:], x_ap[t * 128:(t + 1) * 128, :])
                xts.append(xt)
            gg_sb = xt_pool.tile([1, H], F32)
            nc.sync.dma_start(gg_sb[:], gg_ap[:])
            gdc_sb = cp.tile([128, IK], F32)
            nc.sync.dma_start(gdc_sb[:], gd_ap[:])

            # ---- inits (before any collective hits the gpsimd queue) --
            ones = cp.tile([128, 1], F32)
            nc.gpsimd.memset(ones[:], 1.0)
            epsb = cp.tile([128, 1], F32)
            nc.gpsimd.memset(epsb[:], EPS)
            ident_b = cp.tile([128, 128], BF16)
            make_identity(nc, ident_b[:])
            ident_f = cp.tile([128, 128], F32)
            make_identity(nc, ident_f[:])

            # warmup collective: the first collective after the init
            # barrier pays ~20us extra; burn that on a dummy.
            warm_sb = cp.tile([1, 1], F32)
            nc.vector.memset(warm_sb[:], 0.0)
            warm_in = dram.tile([1, 1], F32, name="warm_in")
            warm_out = dram.tile([1, 1], F32, addr_space="Shared",
                                 name="warm_out")
            nc.sync.dma_start(warm_in[:], warm_sb[:])
            nc.gpsimd.collective_compute("AllGather", Alu.bypass,
                                         replica_groups=rg,
                                         ins=[warm_in[:]],
                                         outs=[dram.tile([8, 1], F32,
                                               addr_space="Shared",
                                               name="warm_o8")[:]])
            del warm_out

            # ---- P0x: x rmsnorm + int8-grid quant + transpose ----
            # first in the DVE FIFO so nothing weight-side delays xqT.
            xq_p = tc.tile_pool(name="xqp", bufs=1)
            xq_pool = xq_p.__enter__()
            g_bc = xq_pool.tile([128, H], F32)
            nc.gpsimd.partition_broadcast(g_bc[:], gg_sb[:])
            xqT = xq_pool.tile([128, HK * TPC], BF16)
            amax1 = colp.tile([128, TT], F32)
            with tc.tile_pool(name="xwork", bufs=2) as xw, \
                 tc.tile_pool(name="psX", bufs=2, space="PSUM") as psX:
                for t in range(TT):
                    xt = xts[t]
                    xsq = xw.tile([128, H], F32, tag="xsq", name=f"xsq{t}")
                    ssq = colp.tile([128, 1], F32, name=f"ssq{t}")
                    nc.scalar.activation(xsq[:], xt[:], Act.Square,
                                         accum_out=ssq[:])
                    sd = colp.tile([128, 1], F32, name=f"sd{t}")
                    nc.scalar.activation(sd[:], ssq[:], Act.Sqrt, bias=epsb[:],
                                         scale=1.0 / H)
                    rstd = colp.tile([128, 1], F32, name=f"rstd{t}")
                    nc.vector.reciprocal(rstd[:], sd[:])
                    # ACT applies the per-token rstd; DVE only does the
                    # g-mult and the amax reduce (keeps DVE queue short)
                    xr = xw.tile([128, H], F32, tag="xr", name=f"xr{t}")
                    nc.scalar.activation(xr[:], xt[:], Act.Copy,
                                         scale=rstd[:])
                    xn = xw.tile([128, H], F32, tag="xn", name=f"xn{t}")
                    nc.vector.tensor_tensor(xn[:], xr[:], g_bc[:], Alu.mult)
                    am = amax1[:, t:t + 1]
                    nc.vector.tensor_reduce(am, xn[:], axis=X, op=Alu.max,
                                            apply_absolute_value=True)
                    nc.vector.tensor_scalar_max(am, am, EPS)
                    rc = colp.tile([128, 1], F32, name=f"rc{t}")
                    nc.vector.reciprocal(rc[:], am)
                    s1 = colp.tile([128, 1], F32, name=f"s1{t}")
                    nc.vector.tensor_scalar_mul(s1[:], rc[:], 127.0)
                    # int8-grid round via the magic-constant trick on ACT
                    q1 = xw.tile([128, H], F32, tag="q1", name=f"q1{t}")
                    nc.scalar.activation(q1[:], xn[:], Act.Copy,
                                         scale=s1[:], bias=C_MAGIC)
                    q = xw.tile([128, H], BF16, tag="q", name=f"q{t}")
                    nc.scalar.activation(q[:], q1[:], Act.Copy,
                                         bias=-C_MAGIC)
                    for i in range(HK):
                        tps = psX.tile([128, 128], BF16, tag="tps",
                                       name=f"tps{t}_{i}")
                        nc.tensor.transpose(tps[:], q[:, i * 128:(i + 1) * 128],
                                            ident_b[:])
                        nc.scalar.copy(xqT[:, i * TPC + t * 128:
                                           i * TPC + (t + 1) * 128], tps[:])

            # ---- P0w: sharded abs-sums; ONE AllGather + on-PE sum ----
            stat_sb = colp.tile([1, 2], F32)
            with tc.tile_pool(name="statp", bufs=3) as stp, \
                 tc.tile_pool(name="psStat", bufs=1, space="PSUM") as psS:
                def stat_pass(w_ap, nblk, ncol, key, col):
                    parts = colp.tile([128, nblk * ncol], F32, name=f"pt{key}")
                    for blk in range(nblk):
                        for ck in range(ncol):
                            wch = stp.tile([128, CCHUNK], F32, tag="wch",
                                           name=f"w{key}{blk}_{ck}")
                            nc.sync.dma_start(
                                wch[:], w_ap[blk * 128:(blk + 1) * 128,
                                             ck * CCHUNK:(ck + 1) * CCHUNK])
                            nc.vector.tensor_reduce(
                                parts[:, blk * ncol + ck:blk * ncol + ck + 1],
                                wch[:], axis=X, op=Alu.add,
                                apply_absolute_value=True)
                    sums = colp.tile([128, 1], F32, name=f"sm{key}")
                    nc.vector.tensor_reduce(sums[:], parts[:], axis=X,
                                            op=Alu.add)
                    ps = psS.tile([1, 1], F32, tag=f"sps{key}",
                                  name=f"sps{key}")
                    nc.tensor.matmul(ps[:], sums[:], ones[:], start=True,
                                     stop=True)
                    nc.scalar.copy(stat_sb[:, col:col + 1], ps[:])

                stat_pass(wg_ap, 2, O2 // CCHUNK, "g", 0)
                stat_pass(wd_ap, 8, 1, "d", 1)

                sin = dram.tile([1, 2], F32, name="statin")
                sout = dram.tile([8, 2], F32, addr_space="Shared",
                                 name="statout")
                nc.sync.dma_start(sin[:], stat_sb[:])
                nc.gpsimd.collective_compute(
                    "AllGather", Alu.bypass, replica_groups=rg,
                    ins=[sin[:]], outs=[sout[:]])
                res8 = colp.tile([8, 2], F32, name="res8")
                nc.sync.dma_start(res8[:], sout[:])
                ps2 = psS.tile([1, 2], F32, tag="spsum", name="spsum")
                nc.tensor.matmul(ps2[:], ones[0:8, 0:1], res8[:],
                                 start=True, stop=True)
                stat_res = colp.tile([1, 2], F32, name="statres")
                nc.scalar.copy(stat_res[:], ps2[:])

            def cols_to_row_bcast(cols, name):
                with tc.tile_pool(name=f"psR{name}", bufs=1,
                                  space="PSUM") as psR:
                    ps = psR.tile([TT, 128], F32, tag="rowps",
                                  name=f"{name}_ps")
                    nc.tensor.transpose(ps[:], cols[:], ident_f[:])
                    r4 = colp.tile([TT, 128], F32, name=f"{name}_r4")
                    nc.scalar.copy(r4[:], ps[:])
                # bounce [4,128] -> [1,512] through DRAM (linear reinterp)
                rb = dram.tile([TT, 128], F32, name=f"{name}_rb")
                nc.sync.dma_start(rb[:], r4[:])
                row = colp.tile([1, TPC], F32, name=f"{name}_r")
                nc.sync.dma_start(row[:],
                                  rb[:].rearrange("a b -> (a b)").rearrange("(o f) -> o f", o=1))
                bc = colp.tile([128, TPC], F32, name=f"{name}_bc")
                nc.gpsimd.partition_broadcast(bc[:], row[:])
                return bc

            # ---- thresholds from the AllGathered stats ----
            def bcast_scaled(src, scale, name):
                t1 = colp.tile([1, 1], F32, name=f"{name}_s")
                nc.vector.tensor_scalar_mul(t1[:], src, scale)
                t2 = colp.tile([128, 1], F32, name=f"{name}_b")
                nc.gpsimd.partition_broadcast(t2[:], t1[:])
                return t2

            thr_g = bcast_scaled(stat_res[0:1, 0:1], 2.0 ** -26, "thrg")
            m_g = bcast_scaled(stat_res[0:1, 0:1], 2.0 ** -25, "mg")
            thr_d = bcast_scaled(stat_res[0:1, 1:2], 2.0 ** -25, "thrd")
            m_d = bcast_scaled(stat_res[0:1, 1:2], 2.0 ** -24, "md")
            nthr_g = colp.tile([128, 1], F32)
            nc.vector.tensor_scalar_mul(nthr_g[:], thr_g[:], -1.0)
            nthr_d = colp.tile([128, 1], F32)
            nc.vector.tensor_scalar_mul(nthr_d[:], thr_d[:], -1.0)

            ys_cols = colp.tile([128, TT], F32)
            nc.vector.tensor_scalar(ys_cols[:], amax1[:], m_g[:], 1.0 / 127.0,
                                    Alu.mult, Alu.mult)
            ys_bc = cols_to_row_bcast(ys_cols, "ys")

            # ---- P2: ternarize (fp8). chunks 0-1 now; 2-15 and the
            # down-proj emitted inside the mm1 loop. ----
            tg_shs = [dram.tile([GSH, 2 * GW], F8, name=f"tgsh{k}")
                      for k in range(NCH)]
            tg_fulls = [dram.tile([H, 2 * GW], F8, addr_space="Shared",
                                  name=f"tgf{k}") for k in range(NCH)]
            # down-proj split into two H-half AGs (A: cols 0-1023 for
            # mm2 half 0; B: cols 1024-2047 for half 1)
            td_shA = dram.tile([DSH, H // 2], F8, name="tdshA")
            td_shB = dram.tile([DSH, H // 2], F8, name="tdshB")
            td_fullA = dram.tile([I, H // 2], F8, addr_space="Shared",
                                 name="tdfA")
            td_fullB = dram.tile([I, H // 2], F8, addr_space="Shared",
                                 name="tdfB")

            tp_ctx = tc.tile_pool(name="ternp", bufs=2)
            tp = tp_ctx.__enter__()

            def tern_cols(eng, w_ap, blk, src_c0, ncol, thr, nthr, dsts, nm):
                # dsts: list of (dst_tile, dst_c0, width) consuming the
                # ternarized row left-to-right
                w = tp.tile([128, ncol], F32, tag="tw", name=f"tw{nm}")
                eng.dma_start(w[:], w_ap[blk * 128:(blk + 1) * 128,
                                         src_c0:src_c0 + ncol])
                a = tp.tile([128, ncol], BF16, tag="ta", name=f"ta{nm}")
                nc.vector.tensor_scalar(a[:], w[:], thr[:], 0.5,
                                        Alu.is_gt, Alu.subtract)
                b = tp.tile([128, ncol], BF16, tag="tb", name=f"tb{nm}")
                nc.vector.tensor_scalar(b[:], w[:], nthr[:], 0.5,
                                        Alu.is_ge, Alu.subtract)
                t = tp.tile([128, ncol], F8, tag="tc", name=f"tc{nm}")
                nc.vector.tensor_tensor(t[:], a[:], b[:], Alu.add)
                off = 0
                for dst, dst_c0, width in dsts:
                    eng.dma_start(dst[blk * 128:(blk + 1) * 128,
                                      dst_c0:dst_c0 + width],
                                  t[:, off:off + width])
                    off += width

            def tern_gate_chunk(k, eng):
                for blk in range(GSH // 128):
                    tern_cols(eng, wg_ap, blk, k * GW, GW, thr_g, nthr_g,
                              [(tg_shs[k], 0, GW)], f"g{k}_{blk}")
                    tern_cols(eng, wg_ap, blk, I + k * GW, GW, thr_g, nthr_g,
                              [(tg_shs[k], GW, GW)], f"v{k}_{blk}")
                nc.gpsimd.collective_compute(
                    "AllGather", Alu.bypass, replica_groups=rg,
                    ins=[tg_shs[k][:]], outs=[tg_fulls[k][:]])

            tern_gate_chunk(0, nc.sync)
            tern_gate_chunk(1, nc.sync)
            tern_gate_chunk(2, nc.sync)

            # ---- P3: mm1 + SwiGLU -> h [I, tok] f32 spilled to DRAM,
            # with tern chunks 2-15 + down-proj tern emitted inside ----
            h_dram = dram.tile([I, TPC], F32)
            acc_sq = colp.tile([128, TPC], F32)
            nc.vector.memset(acc_sq[:], 0.0)
            acc_mxp = colp.tile([128, TPC], F32)
            nc.vector.memset(acc_mxp[:], -3.0e38)
            acc_mxn = colp.tile([128, TPC], F32)
            nc.vector.memset(acc_mxn[:], 3.0e38)
            tgvs = [t[:].rearrange("(i p) o -> p i o", p=128)
                    for t in tg_fulls]

            def tern_td_block(blk):
                # one [128, H] row-block, halves written to A/B tiles
                tern_cols(nc.gpsimd, wd_ap, blk, 0, H, thr_d, nthr_d,
                          [(td_shA, 0, H // 2), (td_shB, 0, H // 2)],
                          f"d{blk}")

            with tc.tile_pool(name="p3", bufs=3) as p3, \
                 tc.tile_pool(name="psMM1", bufs=2, space="PSUM") as psM1:
              for sb in range(16):  # o-blocks of 512 cols; chunk sb 1:1
                tg_g = p3.tile([128, HK, 512], F8, tag="tg_g",
                               name=f"tgg{sb}")
                nc.sync.dma_start(tg_g[:], tgvs[sb][:, :, 0:GW])
                tg_v = p3.tile([128, HK, 512], F8, tag="tg_v",
                               name=f"tgv{sb}")
                nc.sync.dma_start(tg_v[:], tgvs[sb][:, :, GW:2 * GW])
                for si in range(2):
                    s = sb * 2 + si
                    pg = [psM1.tile([128, TPC], F32, tag=f"pg{jj}",
                                    name=f"pg{s}_{jj}") for jj in range(2)]
                    pv = [psM1.tile([128, TPC], F32, tag=f"pv{jj}",
                                    name=f"pv{s}_{jj}") for jj in range(2)]
                    for i in range(HK):
                        rhs = xqT[:, i * TPC:(i + 1) * TPC]
                        st, sp = i == 0, i == HK - 1
                        for jj in range(2):
                            co = si * 256 + jj * 128
                            nc.tensor.matmul(
                                pg[jj][:], tg_g[:, i, co:co + 128],
                                rhs, start=st, stop=sp)
                            nc.tensor.matmul(
                                pv[jj][:], tg_v[:, i, co:co + 128],
                                rhs, start=st, stop=sp)
                    for jj in range(2):
                        jb = sb * 4 + si * 2 + jj
                        gsc = p3.tile([128, TPC], F32, tag="gsc", name=f"gs{jb}")
                        nc.vector.tensor_tensor(gsc[:], pg[jj][:], ys_bc[:],
                                                Alu.mult)
                        sg = p3.tile([128, TPC], F32, tag="sg", name=f"sg{jb}")
                        nc.scalar.activation(sg[:], gsc[:], Act.Silu)
                        vsc = p3.tile([128, TPC], F32, tag="vsc", name=f"vs{jb}")
                        nc.vector.tensor_tensor(vsc[:], pv[jj][:], ys_bc[:],
                                                Alu.mult)
                        hj = p3.tile([128, TPC], F32, tag="hj", name=f"hj{jb}")
                        nc.vector.tensor_tensor(hj[:], sg[:], vsc[:], Alu.mult)
                        nc.sync.dma_start(h_dram[jb * 128:(jb + 1) * 128, :],
                                          hj[:])
                        hsq = p3.tile([128, TPC], F32, tag="hsq", name=f"hq{jb}")
                        nc.scalar.activation(hsq[:], hj[:], Act.Square)
                        nc.vector.tensor_tensor(acc_sq[:], acc_sq[:], hsq[:],
                                                Alu.add)
                        hg = p3.tile([128, TPC], F32, tag="hg", name=f"hg{jb}")
                        nc.vector.tensor_scalar_mul(hg[:], hj[:],
                                                    gdc_sb[:, jb:jb + 1])
                        nc.vector.tensor_tensor(acc_mxp[:], acc_mxp[:], hg[:],
                                                Alu.max)
                        nc.vector.tensor_tensor(acc_mxn[:], acc_mxn[:], hg[:],
                                                Alu.min)
                # interleaved tern emission: gate chunk sb+3 right after
                # sb (consumed 3 sb later, ~30us of margin); one
                # down-proj row-block after sb 3-10. The down-proj AGs
                # trigger after the LAST gate chunk so they never block
                # a gate chunk on the serial cc stream: A after sb 12,
                # B after sb 13 (mm2 half 0 needs A ~60us later).
                if sb + 3 <= NCH - 1:
                    tern_gate_chunk(sb + 3, nc.gpsimd)
                if 3 <= sb <= 10:
                    tern_td_block(sb - 3)
                if sb == 12:
                    nc.gpsimd.collective_compute(
                        "AllGather", Alu.bypass, replica_groups=rg,
                        ins=[td_shA[:]], outs=[td_fullA[:]])
                if sb == 13:
                    nc.gpsimd.collective_compute(
                        "AllGather", Alu.bypass, replica_groups=rg,
                        ins=[td_shB[:]], outs=[td_fullB[:]])

            tp_ctx.__exit__(None, None, None)
            xq_p.__exit__(None, None, None)
            xt_p.__exit__(None, None, None)

            # ---- P4: per-token stats over I ----
            qs_cols = colp.tile([128, TT], F32)
            y2s_cols = colp.tile([128, TT], F32)
            ps4_ctx = tc.tile_pool(name="ps4", bufs=1, space="PSUM")
            ps4 = ps4_ctx.__enter__()
            for t in range(TT):
                sl = slice(t * 128, (t + 1) * 128)
                pssq = ps4.tile([128, 128], F32, tag="pssq", name=f"pq{t}")
                nc.tensor.transpose(pssq[:], acc_sq[:, sl], ident_f[:])
                ss = colp.tile([128, 1], F32, name=f"hss{t}")
                nc.vector.tensor_reduce(ss[:], pssq[:], axis=X, op=Alu.add)
                psm1 = ps4.tile([128, 128], F32, tag="psm1", name=f"pm1{t}")
                nc.tensor.transpose(psm1[:], acc_mxp[:, sl], ident_f[:])
                c1 = colp.tile([128, 1], F32, name=f"hc1{t}")
                nc.vector.tensor_reduce(c1[:], psm1[:], axis=X, op=Alu.max,
                                        apply_absolute_value=True)
                psm2 = ps4.tile([128, 128], F32, tag="psm2", name=f"pm2{t}")
                nc.tensor.transpose(psm2[:], acc_mxn[:, sl], ident_f[:])
                c2 = colp.tile([128, 1], F32, name=f"hc2{t}")
                nc.vector.tensor_reduce(c2[:], psm2[:], axis=X, op=Alu.max,
                                        apply_absolute_value=True)
                amax_hg = colp.tile([128, 1], F32, name=f"amhg{t}")
                nc.vector.tensor_tensor(amax_hg[:], c1[:], c2[:], Alu.max)
                sd2 = colp.tile([128, 1], F32, name=f"sd2{t}")
                nc.scalar.activation(sd2[:], ss[:], Act.Sqrt, bias=epsb[:],
                                     scale=1.0 / I)
                rstd2 = colp.tile([128, 1], F32, name=f"rstd2{t}")
                nc.vector.reciprocal(rstd2[:], sd2[:])
                t1 = colp.tile([128, 1], F32, name=f"t1{t}")
                nc.vector.tensor_scalar(t1[:], amax_hg[:], rstd2[:], EPS,
                                        Alu.mult, Alu.max)
                rc2 = colp.tile([128, 1], F32, name=f"rc2{t}")
                nc.vector.reciprocal(rc2[:], t1[:])
                s2 = colp.tile([128, 1], F32, name=f"s2{t}")
                nc.vector.tensor_scalar_mul(s2[:], rc2[:], 127.0)
                nc.vector.tensor_scalar_mul(qs_cols[:, t:t + 1], rstd2[:],
                                            s2[:])
                nc.vector.tensor_scalar(y2s_cols[:, t:t + 1], t1[:], m_d[:],
                                        1.0 / 127.0, Alu.mult, Alu.mult)
            ps4_ctx.__exit__(None, None, None)
            qs_bc = cols_to_row_bcast(qs_cols, "qs")

            # ---- P5: j-outer mm2, 2 H-halves; quant fused in half 0,
            # q2T cached in SBUF (bf16) for half 1 ----
            tdvA = td_fullA[:].rearrange("(j p) o -> p j o", p=128)
            tdvB = td_fullB[:].rearrange("(j p) o -> p j o", p=128)
            q2p_ctx = tc.tile_pool(name="q2p", bufs=1)
            q2p = q2p_ctx.__enter__()
            q2T = q2p.tile([128, IK * TPC], BF16)
            with tc.tile_pool(name="p5", bufs=3) as p5, \
                 tc.tile_pool(name="psMM2", bufs=1, space="PSUM") as psM2:
                for half in range(2):
                    tdv = tdvA if half == 0 else tdvB
                    p2 = [psM2.tile([128, 512], F32, tag=f"p2_{q}",
                                    name=f"p2_{half}_{q}") for q in range(8)]
                    for j in range(IK):
                        if half == 0:
                            hjl = p5.tile([128, TPC], F32, tag="hjl",
                                          name=f"h4_{j}")
                            nc.sync.dma_start(
                                hjl[:], h_dram[j * 128:(j + 1) * 128, :])
                            hg2 = p5.tile([128, TPC], F32, tag="hg2",
                                          name=f"g4_{j}")
                            nc.scalar.activation(hg2[:], hjl[:], Act.Copy,
                                                 scale=gdc_sb[:, j:j + 1])
                            nc.vector.tensor_tensor(hg2[:], hg2[:], qs_bc[:],
                                                    Alu.mult)
                            nc.vector.tensor_scalar(
                                q2T[:, j * TPC:(j + 1) * TPC], hg2[:],
                                C_MAGIC, C_MAGIC, Alu.add, Alu.subtract)
                        td_j = p5.tile([128, 1024], F8, tag="td",
                                       name=f"td{half}_{j}")
                        nc.scalar.dma_start(td_j[:], tdv[:, j, :])
                        st, sp = j == 0, j == IK - 1
                        for t in range(TT):
                            for hq in range(2):
                                nc.tensor.matmul(
                                    p2[t * 2 + hq][:],
                                    q2T[:, j * TPC + t * 128:
                                        j * TPC + (t + 1) * 128],
                                    td_j[:, hq * 512:(hq + 1) * 512],
                                    start=st, stop=sp)
                    for t in range(TT):
                        for hq in range(2):
                            yt = p5.tile([128, 512], F32, tag="yt",
                                         name=f"yt{half}_{t}_{hq}")
                            nc.vector.tensor_scalar_mul(
                                yt[:], p2[t * 2 + hq][:],
                                y2s_cols[:, t:t + 1])
                            nc.sync.dma_start(
                                y_ap[t * 128:(t + 1) * 128,
                                     half * 1024 + hq * 512:
                                     half * 1024 + (hq + 1) * 512], yt[:])
            q2p_ctx.__exit__(None, None, None)
    return nc


_CACHE = {}


def _get_compiled():
    if "nc" not in _CACHE:
        nc = bacc.Bacc("TRN2", target_bir_lowering=False, debug=False,
                       enable_asserts=False, num_devices=NC_N)
        build(nc)
        nc.compile()
        _CACHE["nc"] = nc
    return _CACHE["nc"]


def kernel(x, w_gate, g_gate, w_down, g_down):
    nc = _get_compiled()
    x2 = np.ascontiguousarray(np.asarray(x, np.float32).reshape(TOK, H))
    wgT = np.asarray(w_gate, np.float32).T
    wdT = np.asarray(w_down, np.float32).T
    gg = np.ascontiguousarray(np.asarray(g_gate, np.float32).reshape(1, H))
    gdc = np.ascontiguousarray(
        np.asarray(g_down, np.float32).reshape(IK, 128).T)
    in_maps = []
    for c in range(NC_N):
        in_maps.append({
            "x": x2[c * TPC:(c + 1) * TPC],
            "wgt": np.ascontiguousarray(wgT[c * GSH:(c + 1) * GSH]),
            "wdt": np.ascontiguousarray(wdT[c * DSH:(c + 1) * DSH]),
            "gg": gg,
            "gdc": gdc,
        })
    res = run_bass_kernel_spmd(nc, in_maps, core_ids=list(range(NC_N)))
    out = np.concatenate([res.results[c]["y"] for c in range(NC_N)], axis=0)
    return out.reshape(B, S, H).astype(np.float32)


# revision 31
# speedup vs baseline: 1.0209x; 1.0209x over previous
"""HGRNBitMLP (BitNet-style SwiGLU MLP) on 8 TRN2 NeuronCores.

Data-parallel over the 4096 tokens (512/core). Weight ternarization is
sharded 1/8 per core; ternary weights are stored as fp8e4 (exact for
{-1,0,+1}) and AllGathered in fp8 — half the bytes of bf16. The global
mean(|w|) comes from two tiny AllReduces. Activations are quantized to
the int8 grid (exact in bf16), so every matmul is an exact-integer
fp8xbf16 matmul with f32 PSUM accumulation. Per-token scales are
applied outside the matmuls.

Engine-queue discipline (strict FIFO per engine):
 - program order: inits -> x-quant (P0x) -> weight stats/AR -> tern
   chunks 0-1 -> mm1 loop with tern chunks 2-7 and the down-proj tern
   EMITTED INSIDE the loop (so their DVE ops interleave with SwiGLU
   instead of damming the queue).
 - x + gate-tile loads ride the scalar(ACT) / sync HWDGE rings; the
   interleaved tern DMA traffic rides the gpsimd SWDGE ring, keeping
   the sync ring free for mm1's tile loads and h spills.
 - h-square on ACT, stats accumulators on DVE: the gpsimd queue stays
   clear for collective triggers.

Layouts: x is loaded [tok, h], quantized, PE-transposed to xqT [h, tok].
mm1 produces y^T tiles [o, tok]; SwiGLU keeps h as [I, tok] (f32,
spilled to DRAM) so mm2's operand q2T [I, tok] needs no transpose.
mm2 is j-outer over I with quantization fused into the first H-half
(q2T cached in SBUF bf16 for the second half) and computes out
[tok, H] in two H-halves.
"""
import sys

try:
    import concourse  # noqa: F401
except ImportError:
    sys.path.insert(0, "/opt/trn_rl_repo")

import numpy as np

import concourse.tile as tile
from concourse import bacc, mybir
from concourse.bass_utils import run_bass_kernel_spmd
from concourse.masks import make_identity

F32, BF16, F8 = mybir.dt.float32, mybir.dt.bfloat16, mybir.dt.float8e4
Alu = mybir.AluOpType
Act = mybir.ActivationFunctionType
X = mybir.AxisListType.X

NC_N = 8
B, S, H, I = 2, 2048, 2048, 8192
O2 = 2 * I
TOK = B * S
TPC = TOK // NC_N   # 512 tokens/core
TT = TPC // 128     # 4 token tiles
HK = H // 128       # 16 h tiles
IK = I // 128       # 64 I tiles
GSH = H // NC_N     # 256 rows of w_gate^T per core
DSH = I // NC_N     # 1024 rows of w_down^T per core
EPS = 1e-5
C_MAGIC = 12582912.0  # 1.5*2^23; (x+C)-C rounds f32 to nearest-even int
CCHUNK = 2048
NCH = 8             # gate AG chunks; each pairs 1024 gate + 1024 v cols
GW = O2 // NCH // 2  # 1024 och of gate (and of v) per chunk


def build(nc):
    x_ap = nc.dram_tensor("x", [TPC, H], F32, kind="ExternalInput").ap()
    wg_ap = nc.dram_tensor("wgt", [GSH, O2], F32, kind="ExternalInput").ap()
    wd_ap = nc.dram_tensor("wdt", [DSH, H], F32, kind="ExternalInput").ap()
    gg_ap = nc.dram_tensor("gg", [1, H], F32, kind="ExternalInput").ap()
    gd_ap = nc.dram_tensor("gdc", [128, IK], F32, kind="ExternalInput").ap()
    y_ap = nc.dram_tensor("y", [TPC, H], F32, kind="ExternalOutput").ap()
    rg = [list(range(NC_N))]

    with tile.TileContext(nc) as tc:
        with tc.tile_pool(name="dram", bufs=1, space="DRAM") as dram, \
             tc.tile_pool(name="perm", bufs=1) as cp, \
             tc.tile_pool(name="colp", bufs=1) as colp:

            # ---- inits (before any collective hits the gpsimd queue) --
            ones = cp.tile([128, 1], F32)
            nc.gpsimd.memset(ones[:], 1.0)
            epsb = cp.tile([128, 1], F32)
            nc.gpsimd.memset(epsb[:], EPS)
            ident_b = cp.tile([128, 128], BF16)
            make_identity(nc, ident_b[:])
            ident_f = cp.tile([128, 128], F32)
            make_identity(nc, ident_f[:])
            gdc_sb = cp.tile([128, IK], F32)
            nc.scalar.dma_start(gdc_sb[:], gd_ap[:])

            # warmup collective: the first collective after the init
            # barrier pays ~20us extra; burn that on a dummy.
            warm_sb = cp.tile([1, 1], F32)
            nc.vector.memset(warm_sb[:], 0.0)
            warm_in = dram.tile([1, 1], F32, name="warm_in")
            warm_out = dram.tile([8, 1], F32, addr_space="Shared",
                                 name="warm_out")
            nc.sync.dma_start(warm_in[:], warm_sb[:])
            nc.gpsimd.collective_compute("AllGather", Alu.bypass,
                                         replica_groups=rg,
                                         ins=[warm_in[:]], outs=[warm_out[:]])

            # ---- P0x: x rmsnorm + int8-grid quant + transpose ----
            # first in the DVE FIFO so nothing weight-side delays xqT.
            xq_p = tc.tile_pool(name="xqp", bufs=1)
            xq_pool = xq_p.__enter__()
            gg_sb = xq_pool.tile([1, H], F32)
            nc.scalar.dma_start(gg_sb[:], gg_ap[:])
            g_bc = xq_pool.tile([128, H], F32)
            nc.gpsimd.partition_broadcast(g_bc[:], gg_sb[:])
            xqT = xq_pool.tile([128, HK * TPC], BF16)
            amax1 = colp.tile([128, TT], F32)
            with tc.tile_pool(name="xwork", bufs=2) as xw, \
                 tc.tile_pool(name="psX", bufs=2, space="PSUM") as psX:
                for t in range(TT):
                    xt = xw.tile([128, H], F32, tag="xt", name=f"xt{t}")
                    nc.scalar.dma_start(xt[:], x_ap[t * 128:(t + 1) * 128, :])
                    xsq = xw.tile([128, H], F32, tag="xsq", name=f"xsq{t}")
                    ssq = colp.tile([128, 1], F32, name=f"ssq{t}")
                    nc.scalar.activation(xsq[:], xt[:], Act.Square,
                                         accum_out=ssq[:])
                    sd = colp.tile([128, 1], F32, name=f"sd{t}")
                    nc.scalar.activation(sd[:], ssq[:], Act.Sqrt, bias=epsb[:],
                                         scale=1.0 / H)
                    rstd = colp.tile([128, 1], F32, name=f"rstd{t}")
                    nc.vector.reciprocal(rstd[:], sd[:])
                    # ACT applies the per-token rstd; DVE only does the
                    # g-mult and the amax reduce (keeps DVE queue short)
                    xr = xw.tile([128, H], F32, tag="xr", name=f"xr{t}")
                    nc.scalar.activation(xr[:], xt[:], Act.Copy,
                                         scale=rstd[:])
                    xn = xw.tile([128, H], F32, tag="xn", name=f"xn{t}")
                    nc.vector.tensor_tensor(xn[:], xr[:], g_bc[:], Alu.mult)
                    am = amax1[:, t:t + 1]
                    nc.vector.tensor_reduce(am, xn[:], axis=X, op=Alu.max,
                                            apply_absolute_value=True)
                    nc.vector.tensor_scalar_max(am, am, EPS)
                    rc = colp.tile([128, 1], F32, name=f"rc{t}")
                    nc.vector.reciprocal(rc[:], am)
                    s1 = colp.tile([128, 1], F32, name=f"s1{t}")
                    nc.vector.tensor_scalar_mul(s1[:], rc[:], 127.0)
                    # int8-grid round via the magic-constant trick on ACT
                    q1 = xw.tile([128, H], F32, tag="q1", name=f"q1{t}")
                    nc.scalar.activation(q1[:], xn[:], Act.Copy,
                                         scale=s1[:], bias=C_MAGIC)
                    q = xw.tile([128, H], BF16, tag="q", name=f"q{t}")
                    nc.scalar.activation(q[:], q1[:], Act.Copy,
                                         bias=-C_MAGIC)
                    for i in range(HK):
                        tps = psX.tile([128, 128], BF16, tag="tps",
                                       name=f"tps{t}_{i}")
                        nc.tensor.transpose(tps[:], q[:, i * 128:(i + 1) * 128],
                                            ident_b[:])
                        nc.scalar.copy(xqT[:, i * TPC + t * 128:
                                           i * TPC + (t + 1) * 128], tps[:])

            # ---- P0w: sharded abs-sums; ONE tiny AllGather (2.9us vs
            # 34-46us for an AllReduce) + on-PE partition sum ----
            stat_sb = colp.tile([1, 2], F32)
            with tc.tile_pool(name="statp", bufs=3) as stp, \
                 tc.tile_pool(name="psStat", bufs=1, space="PSUM") as psS:
                def stat_pass(w_ap, nblk, ncol, key, col):
                    parts = colp.tile([128, nblk * ncol], F32, name=f"pt{key}")
                    for blk in range(nblk):
                        for ck in range(ncol):
                            wch = stp.tile([128, CCHUNK], F32, tag="wch",
                                           name=f"w{key}{blk}_{ck}")
                            nc.sync.dma_start(
                                wch[:], w_ap[blk * 128:(blk + 1) * 128,
                                             ck * CCHUNK:(ck + 1) * CCHUNK])
                            nc.vector.tensor_reduce(
                                parts[:, blk * ncol + ck:blk * ncol + ck + 1],
                                wch[:], axis=X, op=Alu.add,
                                apply_absolute_value=True)
                    sums = colp.tile([128, 1], F32, name=f"sm{key}")
                    nc.vector.tensor_reduce(sums[:], parts[:], axis=X,
                                            op=Alu.add)
                    ps = psS.tile([1, 1], F32, tag=f"sps{key}",
                                  name=f"sps{key}")
                    nc.tensor.matmul(ps[:], sums[:], ones[:], start=True,
                                     stop=True)
                    nc.scalar.copy(stat_sb[:, col:col + 1], ps[:])

                stat_pass(wg_ap, 2, O2 // CCHUNK, "g", 0)
                stat_pass(wd_ap, 8, 1, "d", 1)

                sin = dram.tile([1, 2], F32, name="statin")
                sout = dram.tile([8, 2], F32, addr_space="Shared",
                                 name="statout")
                nc.sync.dma_start(sin[:], stat_sb[:])
                nc.gpsimd.collective_compute(
                    "AllGather", Alu.bypass, replica_groups=rg,
                    ins=[sin[:]], outs=[sout[:]])
                res8 = colp.tile([8, 2], F32, name="res8")
                nc.sync.dma_start(res8[:], sout[:])
                ps2 = psS.tile([1, 2], F32, tag="spsum", name="spsum")
                nc.tensor.matmul(ps2[:], ones[0:8, 0:1], res8[:],
                                 start=True, stop=True)
                stat_res = colp.tile([1, 2], F32, name="statres")
                nc.scalar.copy(stat_res[:], ps2[:])

            def cols_to_row_bcast(cols, name):
                with tc.tile_pool(name=f"psR{name}", bufs=1,
                                  space="PSUM") as psR:
                    ps = psR.tile([TT, 128], F32, tag="rowps",
                                  name=f"{name}_ps")
                    nc.tensor.transpose(ps[:], cols[:], ident_f[:])
                    r4 = colp.tile([TT, 128], F32, name=f"{name}_r4")
                    nc.scalar.copy(r4[:], ps[:])
                # bounce [4,128] -> [1,512] through DRAM (linear reinterp)
                rb = dram.tile([TT, 128], F32, name=f"{name}_rb")
                nc.sync.dma_start(rb[:], r4[:])
                row = colp.tile([1, TPC], F32, name=f"{name}_r")
                nc.sync.dma_start(row[:],
                                  rb[:].rearrange("a b -> (a b)").rearrange("(o f) -> o f", o=1))
                bc = colp.tile([128, TPC], F32, name=f"{name}_bc")
                nc.gpsimd.partition_broadcast(bc[:], row[:])
                return bc

            # ---- thresholds from the AllReduced stats ----
            def bcast_scaled(src, scale, name):
                t1 = colp.tile([1, 1], F32, name=f"{name}_s")
                nc.vector.tensor_scalar_mul(t1[:], src, scale)
                t2 = colp.tile([128, 1], F32, name=f"{name}_b")
                nc.gpsimd.partition_broadcast(t2[:], t1[:])
                return t2

            thr_g = bcast_scaled(stat_res[0:1, 0:1], 2.0 ** -26, "thrg")
            m_g = bcast_scaled(stat_res[0:1, 0:1], 2.0 ** -25, "mg")
            thr_d = bcast_scaled(stat_res[0:1, 1:2], 2.0 ** -25, "thrd")
            m_d = bcast_scaled(stat_res[0:1, 1:2], 2.0 ** -24, "md")
            nthr_g = colp.tile([128, 1], F32)
            nc.vector.tensor_scalar_mul(nthr_g[:], thr_g[:], -1.0)
            nthr_d = colp.tile([128, 1], F32)
            nc.vector.tensor_scalar_mul(nthr_d[:], thr_d[:], -1.0)

            ys_cols = colp.tile([128, TT], F32)
            nc.vector.tensor_scalar(ys_cols[:], amax1[:], m_g[:], 1.0 / 127.0,
                                    Alu.mult, Alu.mult)
            ys_bc = cols_to_row_bcast(ys_cols, "ys")

            # ---- P2: ternarize (fp8). chunks 0-1 now; 2-7 and the
            # down-proj emitted inside the mm1 loop. ----
            tg_shs = [dram.tile([GSH, 2 * GW], F8, name=f"tgsh{k}")
                      for k in range(NCH)]
            tg_fulls = [dram.tile([H, 2 * GW], F8, addr_space="Shared",
                                  name=f"tgf{k}") for k in range(NCH)]
            td_sh = dram.tile([DSH, H], F8)
            td_full = dram.tile([I, H], F8, addr_space="Shared")

            tp_ctx = tc.tile_pool(name="ternp", bufs=2)
            tp = tp_ctx.__enter__()

            def tern_cols(eng, w_ap, blk, src_c0, ncol, thr, nthr, dst,
                          dst_c0, nm):
                w = tp.tile([128, ncol], F32, tag="tw", name=f"tw{nm}")
                eng.dma_start(w[:], w_ap[blk * 128:(blk + 1) * 128,
                                         src_c0:src_c0 + ncol])
                a = tp.tile([128, ncol], BF16, tag="ta", name=f"ta{nm}")
                nc.vector.tensor_scalar(a[:], w[:], thr[:], 0.5,
                                        Alu.is_gt, Alu.subtract)
                b = tp.tile([128, ncol], BF16, tag="tb", name=f"tb{nm}")
                nc.vector.tensor_scalar(b[:], w[:], nthr[:], 0.5,
                                        Alu.is_ge, Alu.subtract)
                t = tp.tile([128, ncol], F8, tag="tc", name=f"tc{nm}")
                nc.vector.tensor_tensor(t[:], a[:], b[:], Alu.add)
                eng.dma_start(dst[blk * 128:(blk + 1) * 128,
                                  dst_c0:dst_c0 + ncol], t[:])

            def tern_gate_chunk(k, eng):
                for blk in range(GSH // 128):
                    tern_cols(eng, wg_ap, blk, k * GW, GW,
                              thr_g, nthr_g, tg_shs[k], 0, f"g{k}_{blk}")
                    tern_cols(eng, wg_ap, blk, I + k * GW, GW,
                              thr_g, nthr_g, tg_shs[k], GW, f"v{k}_{blk}")
                nc.gpsimd.collective_compute(
                    "AllGather", Alu.bypass, replica_groups=rg,
                    ins=[tg_shs[k][:]], outs=[tg_fulls[k][:]])

            tern_gate_chunk(0, nc.sync)
            tern_gate_chunk(1, nc.sync)

            # ---- P3: mm1 + SwiGLU -> h [I, tok] f32 spilled to DRAM,
            # with tern chunks 2-7 + down-proj tern emitted inside ----
            h_dram = dram.tile([I, TPC], F32)
            acc_sq = colp.tile([128, TPC], F32)
            nc.vector.memset(acc_sq[:], 0.0)
            acc_mxp = colp.tile([128, TPC], F32)
            nc.vector.memset(acc_mxp[:], -3.0e38)
            acc_mxn = colp.tile([128, TPC], F32)
            nc.vector.memset(acc_mxn[:], 3.0e38)
            tgvs = [t[:].rearrange("(i p) o -> p i o", p=128)
                    for t in tg_fulls]

            def tern_td_part(part):
                # 8 row-blocks split 2/2/2/2 across parts 0-3
                for blk in range(part * 2, part * 2 + 2):
                    tern_cols(nc.gpsimd, wd_ap, blk, 0, H, thr_d, nthr_d,
                              td_sh, 0, f"d{blk}")
                if part == 3:
                    nc.gpsimd.collective_compute(
                        "AllGather", Alu.bypass, replica_groups=rg,
                        ins=[td_sh[:]], outs=[td_full[:]])

            with tc.tile_pool(name="p3", bufs=3) as p3, \
                 tc.tile_pool(name="psMM1", bufs=2, space="PSUM") as psM1:
              for sb in range(16):  # o-blocks of 512 cols per half
                kg, og = sb // 2, (sb % 2) * 512
                tg_g = p3.tile([128, HK, 512], F8, tag="tg_g",
                               name=f"tgg{sb}")
                nc.sync.dma_start(tg_g[:], tgvs[kg][:, :, og:og + 512])
                tg_v = p3.tile([128, HK, 512], F8, tag="tg_v",
                               name=f"tgv{sb}")
                nc.sync.dma_start(tg_v[:],
                                  tgvs[kg][:, :, GW + og:GW + og + 512])
                for si in range(2):
                    s = sb * 2 + si
                    pg = [psM1.tile([128, TPC], F32, tag=f"pg{jj}",
                                    name=f"pg{s}_{jj}") for jj in range(2)]
                    pv = [psM1.tile([128, TPC], F32, tag=f"pv{jj}",
                                    name=f"pv{s}_{jj}") for jj in range(2)]
                    for i in range(HK):
                        rhs = xqT[:, i * TPC:(i + 1) * TPC]
                        st, sp = i == 0, i == HK - 1
                        for jj in range(2):
                            co = si * 256 + jj * 128
                            nc.tensor.matmul(
                                pg[jj][:], tg_g[:, i, co:co + 128],
                                rhs, start=st, stop=sp)
                            nc.tensor.matmul(
                                pv[jj][:], tg_v[:, i, co:co + 128],
                                rhs, start=st, stop=sp)
                    for jj in range(2):
                        # I-row block: chunk kg covers rows
                        # [kg*1024,(kg+1)*1024); offset og+si*256+jj*128
                        jb = kg * 8 + (sb % 2) * 4 + si * 2 + jj
                        gsc = p3.tile([128, TPC], F32, tag="gsc", name=f"gs{jb}")
                        nc.vector.tensor_tensor(gsc[:], pg[jj][:], ys_bc[:],
                                                Alu.mult)
                        sg = p3.tile([128, TPC], F32, tag="sg", name=f"sg{jb}")
                        nc.scalar.activation(sg[:], gsc[:], Act.Silu)
                        vsc = p3.tile([128, TPC], F32, tag="vsc", name=f"vs{jb}")
                        nc.vector.tensor_tensor(vsc[:], pv[jj][:], ys_bc[:],
                                                Alu.mult)
                        hj = p3.tile([128, TPC], F32, tag="hj", name=f"hj{jb}")
                        nc.vector.tensor_tensor(hj[:], sg[:], vsc[:], Alu.mult)
                        nc.sync.dma_start(h_dram[jb * 128:(jb + 1) * 128, :],
                                          hj[:])
                        hsq = p3.tile([128, TPC], F32, tag="hsq", name=f"hq{jb}")
                        nc.scalar.activation(hsq[:], hj[:], Act.Square)
                        nc.vector.tensor_tensor(acc_sq[:], acc_sq[:], hsq[:],
                                                Alu.add)
                        hg = p3.tile([128, TPC], F32, tag="hg", name=f"hg{jb}")
                        nc.vector.tensor_scalar_mul(hg[:], hj[:],
                                                    gdc_sb[:, jb:jb + 1])
                        nc.vector.tensor_tensor(acc_mxp[:], acc_mxp[:], hg[:],
                                                Alu.max)
                        nc.vector.tensor_tensor(acc_mxn[:], acc_mxn[:], hg[:],
                                                Alu.min)
                # interleaved tern emission: gate chunks 2-7 right after
                # sb 0-5 (their AGs must land well before consumption at
                # sb 4,6,..,14); down-proj tern after sb 6-9 so its AG
                # triggers last on the serial cc stream.
                if sb <= 5:
                    tern_gate_chunk(sb + 2, nc.gpsimd)
                if 6 <= sb <= 9:
                    tern_td_part(sb - 6)

            tp_ctx.__exit__(None, None, None)
            xq_p.__exit__(None, None, None)

            # ---- P4: per-token stats over I ----
            qs_cols = colp.tile([128, TT], F32)
            y2s_cols = colp.tile([128, TT], F32)
            ps4_ctx = tc.tile_pool(name="ps4", bufs=1, space="PSUM")
            ps4 = ps4_ctx.__enter__()
            for t in range(TT):
                sl = slice(t * 128, (t + 1) * 128)
                pssq = ps4.tile([128, 128], F32, tag="pssq", name=f"pq{t}")
                nc.tensor.transpose(pssq[:], acc_sq[:, sl], ident_f[:])
                ss = colp.tile([128, 1], F32, name=f"hss{t}")
                nc.vector.tensor_reduce(ss[:], pssq[:], axis=X, op=Alu.add)
                psm1 = ps4.tile([128, 128], F32, tag="psm1", name=f"pm1{t}")
                nc.tensor.transpose(psm1[:], acc_mxp[:, sl], ident_f[:])
                c1 = colp.tile([128, 1], F32, name=f"hc1{t}")
                nc.vector.tensor_reduce(c1[:], psm1[:], axis=X, op=Alu.max,
                                        apply_absolute_value=True)
                psm2 = ps4.tile([128, 128], F32, tag="psm2", name=f"pm2{t}")
                nc.tensor.transpose(psm2[:], acc_mxn[:, sl], ident_f[:])
                c2 = colp.tile([128, 1], F32, name=f"hc2{t}")
                nc.vector.tensor_reduce(c2[:], psm2[:], axis=X, op=Alu.max,
                                        apply_absolute_value=True)
                amax_hg = colp.tile([128, 1], F32, name=f"amhg{t}")
                nc.vector.tensor_tensor(amax_hg[:], c1[:], c2[:], Alu.max)
                sd2 = colp.tile([128, 1], F32, name=f"sd2{t}")
                nc.scalar.activation(sd2[:], ss[:], Act.Sqrt, bias=epsb[:],
                                     scale=1.0 / I)
                rstd2 = colp.tile([128, 1], F32, name=f"rstd2{t}")
                nc.vector.reciprocal(rstd2[:], sd2[:])
                t1 = colp.tile([128, 1], F32, name=f"t1{t}")
                nc.vector.tensor_scalar(t1[:], amax_hg[:], rstd2[:], EPS,
                                        Alu.mult, Alu.max)
                rc2 = colp.tile([128, 1], F32, name=f"rc2{t}")
                nc.vector.reciprocal(rc2[:], t1[:])
                s2 = colp.tile([128, 1], F32, name=f"s2{t}")
                nc.vector.tensor_scalar_mul(s2[:], rc2[:], 127.0)
                nc.vector.tensor_scalar_mul(qs_cols[:, t:t + 1], rstd2[:],
                                            s2[:])
                nc.vector.tensor_scalar(y2s_cols[:, t:t + 1], t1[:], m_d[:],
                                        1.0 / 127.0, Alu.mult, Alu.mult)
            ps4_ctx.__exit__(None, None, None)
            qs_bc = cols_to_row_bcast(qs_cols, "qs")

            # ---- P5: j-outer mm2, 2 H-halves; quant fused in half 0,
            # q2T cached in SBUF (bf16) for half 1 ----
            tdv = td_full[:].rearrange("(j p) o -> p j o", p=128)
            q2p_ctx = tc.tile_pool(name="q2p", bufs=1)
            q2p = q2p_ctx.__enter__()
            q2T = q2p.tile([128, IK * TPC], BF16)
            with tc.tile_pool(name="p5", bufs=3) as p5, \
                 tc.tile_pool(name="psMM2", bufs=1, space="PSUM") as psM2:
                for half in range(2):
                    p2 = [psM2.tile([128, 512], F32, tag=f"p2_{q}",
                                    name=f"p2_{half}_{q}") for q in range(8)]
                    for j in range(IK):
                        if half == 0:
                            hjl = p5.tile([128, TPC], F32, tag="hjl",
                                          name=f"h4_{j}")
                            nc.sync.dma_start(
                                hjl[:], h_dram[j * 128:(j + 1) * 128, :])
                            hg2 = p5.tile([128, TPC], F32, tag="hg2",
                                          name=f"g4_{j}")
                            nc.vector.tensor_scalar_mul(hg2[:], hjl[:],
                                                        gdc_sb[:, j:j + 1])
                            nc.vector.tensor_tensor(hg2[:], hg2[:], qs_bc[:],
                                                    Alu.mult)
                            nc.vector.tensor_scalar(
                                q2T[:, j * TPC:(j + 1) * TPC], hg2[:],
                                C_MAGIC, C_MAGIC, Alu.add, Alu.subtract)
                        td_j = p5.tile([128, 1024], F8, tag="td",
                                       name=f"td{half}_{j}")
                        nc.scalar.dma_start(
                            td_j[:], tdv[:, j, half * 1024:(half + 1) * 1024])
                        st, sp = j == 0, j == IK - 1
                        for t in range(TT):
                            for hq in range(2):
                                nc.tensor.matmul(
                                    p2[t * 2 + hq][:],
                                    q2T[:, j * TPC + t * 128:
                                        j * TPC + (t + 1) * 128],
                                    td_j[:, hq * 512:(hq + 1) * 512],
                                    start=st, stop=sp)
                    for t in range(TT):
                        for hq in range(2):
                            yt = p5.tile([128, 512], F32, tag="yt",
                                         name=f"yt{half}_{t}_{hq}")
                            nc.vector.tensor_scalar_mul(
                                yt[:], p2[t * 2 + hq][:],
                                y2s_cols[:, t:t + 1])
                            nc.sync.dma_start(
                                y_ap[t * 128:(t + 1) * 128,
                                     half * 1024 + hq * 512:
                                     half * 1024 + (hq + 1) * 512], yt[:])
            q2p_ctx.__exit__(None, None, None)
    return nc


_CACHE = {}


def _get_compiled():
    if "nc" not in _CACHE:
        nc = bacc.Bacc("TRN2", target_bir_lowering=False, debug=False,
                       enable_asserts=False, num_devices=NC_N)
        build(nc)
        nc.compile()
        _CACHE["nc"] = nc
    return _CACHE["nc"]


def kernel(x, w_gate, g_gate, w_down, g_down):
    nc = _get_compiled()
    x2 = np.ascontiguousarray(np.asarray(x, np.float32).reshape(TOK, H))
    wgT = np.asarray(w_gate, np.float32).T
    wdT = np.asarray(w_down, np.float32).T
    gg = np.ascontiguousarray(np.asarray(g_gate, np.float32).reshape(1, H))
    gdc = np.ascontiguousarray(
        np.asarray(g_down, np.float32).reshape(IK, 128).T)
    in_maps = []
    for c in range(NC_N):
        in_maps.append({
            "x": x2[c * TPC:(c + 1) * TPC],
            "wgt": np.ascontiguousarray(wgT[c * GSH:(c + 1) * GSH]),
            "wdt": np.ascontiguousarray(wdT[c * DSH:(c + 1) * DSH]),
            "gg": gg,
            "gdc": gdc,
        })
    res = run_bass_kernel_spmd(nc, in_maps, core_ids=list(range(NC_N)))
    out = np.concatenate([res.results[c]["y"] for c in range(NC_N)], axis=0)
    return out.reshape(B, S, H).astype(np.float32)


# revision 33
# speedup vs baseline: 1.0952x; 1.0728x over previous
"""HGRNBitMLP (BitNet-style SwiGLU MLP) on 8 TRN2 NeuronCores.

Data-parallel over the 4096 tokens (512/core). Weight ternarization is
sharded 1/8 per core; ternary weights are stored as fp8e4 (exact for
{-1,0,+1}) and AllGathered in fp8 — half the bytes of bf16. The global
mean(|w|) comes from two tiny AllReduces. Activations are quantized to
the int8 grid (exact in bf16), so every matmul is an exact-integer
fp8xbf16 matmul with f32 PSUM accumulation. Per-token scales are
applied outside the matmuls.

Engine-queue discipline (strict FIFO per engine):
 - program order: inits -> x-quant (P0x) -> weight stats/AR -> tern
   chunks 0-1 -> mm1 loop with tern chunks 2-7 and the down-proj tern
   EMITTED INSIDE the loop (so their DVE ops interleave with SwiGLU
   instead of damming the queue).
 - x + gate-tile loads ride the scalar(ACT) / sync HWDGE rings; the
   interleaved tern DMA traffic rides the gpsimd SWDGE ring, keeping
   the sync ring free for mm1's tile loads and h spills.
 - h-square on ACT, stats accumulators on DVE: the gpsimd queue stays
   clear for collective triggers.

Layouts: x is loaded [tok, h], quantized, PE-transposed to xqT [h, tok].
mm1 produces y^T tiles [o, tok]; SwiGLU keeps h as [I, tok] (f32,
spilled to DRAM) so mm2's operand q2T [I, tok] needs no transpose.
mm2 is j-outer over I with quantization fused into the first H-half
(q2T cached in SBUF bf16 for the second half) and computes out
[tok, H] in two H-halves.
"""
import sys

try:
    import concourse  # noqa: F401
except ImportError:
    sys.path.insert(0, "/opt/trn_rl_repo")

import numpy as np

import concourse.tile as tile
from concourse import bacc, mybir
from concourse.bass_utils import run_bass_kernel_spmd
from concourse.masks import make_identity

F32, BF16, F8 = mybir.dt.float32, mybir.dt.bfloat16, mybir.dt.float8e4
Alu = mybir.AluOpType
Act = mybir.ActivationFunctionType
X = mybir.AxisListType.X

NC_N = 8
B, S, H, I = 2, 2048, 2048, 8192
O2 = 2 * I
TOK = B * S
TPC = TOK // NC_N   # 512 tokens/core
TT = TPC // 128     # 4 token tiles
HK = H // 128       # 16 h tiles
IK = I // 128       # 64 I tiles
GSH = H // NC_N     # 256 rows of w_gate^T per core
DSH = I // NC_N     # 1024 rows of w_down^T per core
EPS = 1e-5
C_MAGIC = 12582912.0  # 1.5*2^23; (x+C)-C rounds f32 to nearest-even int
CCHUNK = 2048
NCH = 8             # gate AG chunks; each pairs 1024 gate + 1024 v cols
GW = O2 // NCH // 2  # 1024 och of gate (and of v) per chunk


def build(nc):
    x_ap = nc.dram_tensor("x", [TPC, H], F32, kind="ExternalInput").ap()
    wg_ap = nc.dram_tensor("wgt", [GSH, O2], F32, kind="ExternalInput").ap()
    wd_ap = nc.dram_tensor("wdt", [DSH, H], F32, kind="ExternalInput").ap()
    gg_ap = nc.dram_tensor("gg", [1, H], F32, kind="ExternalInput").ap()
    gd_ap = nc.dram_tensor("gdc", [128, IK], F32, kind="ExternalInput").ap()
    y_ap = nc.dram_tensor("y", [TPC, H], F32, kind="ExternalOutput").ap()
    rg = [list(range(NC_N))]

    with tile.TileContext(nc) as tc:
        with tc.tile_pool(name="dram", bufs=1, space="DRAM") as dram, \
             tc.tile_pool(name="perm", bufs=1) as cp, \
             tc.tile_pool(name="colp", bufs=1) as colp:

            # ---- inits (before any collective hits the gpsimd queue) --
            ones = cp.tile([128, 1], F32)
            nc.gpsimd.memset(ones[:], 1.0)
            epsb = cp.tile([128, 1], F32)
            nc.gpsimd.memset(epsb[:], EPS)
            ident_b = cp.tile([128, 128], BF16)
            make_identity(nc, ident_b[:])
            ident_f = cp.tile([128, 128], F32)
            make_identity(nc, ident_f[:])
            gdc_sb = cp.tile([128, IK], F32)
            nc.scalar.dma_start(gdc_sb[:], gd_ap[:])

            # warmup collective: the first collective after the init
            # barrier pays ~20us extra; burn that on a dummy.
            warm_sb = cp.tile([1, 1], F32)
            nc.vector.memset(warm_sb[:], 0.0)
            warm_in = dram.tile([1, 1], F32, name="warm_in")
            warm_out = dram.tile([8, 1], F32, addr_space="Shared",
                                 name="warm_out")
            nc.sync.dma_start(warm_in[:], warm_sb[:])
            nc.gpsimd.collective_compute("AllGather", Alu.bypass,
                                         replica_groups=rg,
                                         ins=[warm_in[:]], outs=[warm_out[:]])

            # ---- P0w: sharded abs-sums; ONE tiny AllGather + on-PE
            # partition sum. First on the DVE FIFO: its reduces are
            # read-paced, so they finish by ~90us and the stats AG
            # fires ~70us earlier than when queued behind P0x. ----
            stat_sb = colp.tile([1, 2], F32)
            with tc.tile_pool(name="statp", bufs=3) as stp, \
                 tc.tile_pool(name="psStat", bufs=1, space="PSUM") as psS:
                def stat_pass(w_ap, nblk, ncol, key, col):
                    parts = colp.tile([128, nblk * ncol], F32, name=f"pt{key}")
                    for blk in range(nblk):
                        for ck in range(ncol):
                            wch = stp.tile([128, CCHUNK], F32, tag="wch",
                                           name=f"w{key}{blk}_{ck}")
                            nc.sync.dma_start(
                                wch[:], w_ap[blk * 128:(blk + 1) * 128,
                                             ck * CCHUNK:(ck + 1) * CCHUNK])
                            nc.vector.tensor_reduce(
                                parts[:, blk * ncol + ck:blk * ncol + ck + 1],
                                wch[:], axis=X, op=Alu.add,
                                apply_absolute_value=True)
                    sums = colp.tile([128, 1], F32, name=f"sm{key}")
                    nc.vector.tensor_reduce(sums[:], parts[:], axis=X,
                                            op=Alu.add)
                    ps = psS.tile([1, 1], F32, tag=f"sps{key}",
                                  name=f"sps{key}")
                    nc.tensor.matmul(ps[:], sums[:], ones[:], start=True,
                                     stop=True)
                    nc.scalar.copy(stat_sb[:, col:col + 1], ps[:])

                stat_pass(wg_ap, 2, O2 // CCHUNK, "g", 0)
                stat_pass(wd_ap, 8, 1, "d", 1)

                sin = dram.tile([1, 2], F32, name="statin")
                sout = dram.tile([8, 2], F32, addr_space="Shared",
                                 name="statout")
                nc.sync.dma_start(sin[:], stat_sb[:])
                nc.gpsimd.collective_compute(
                    "AllGather", Alu.bypass, replica_groups=rg,
                    ins=[sin[:]], outs=[sout[:]])
                res8 = colp.tile([8, 2], F32, name="res8")
                nc.sync.dma_start(res8[:], sout[:])
                ps2 = psS.tile([1, 2], F32, tag="spsum", name="spsum")
                nc.tensor.matmul(ps2[:], ones[0:8, 0:1], res8[:],
                                 start=True, stop=True)
                stat_res = colp.tile([1, 2], F32, name="statres")
                nc.scalar.copy(stat_res[:], ps2[:])

            # ---- P0x: x rmsnorm + int8-grid quant + transpose ----
            xq_p = tc.tile_pool(name="xqp", bufs=1)
            xq_pool = xq_p.__enter__()
            gg_sb = xq_pool.tile([1, H], F32)
            nc.scalar.dma_start(gg_sb[:], gg_ap[:])
            g_bc = xq_pool.tile([128, H], F32)
            nc.gpsimd.partition_broadcast(g_bc[:], gg_sb[:])
            xqT = xq_pool.tile([128, HK * TPC], BF16)
            amax1 = colp.tile([128, TT], F32)
            with tc.tile_pool(name="xwork", bufs=2) as xw, \
                 tc.tile_pool(name="psX", bufs=2, space="PSUM") as psX:
                for t in range(TT):
                    xt = xw.tile([128, H], F32, tag="xt", name=f"xt{t}")
                    nc.scalar.dma_start(xt[:], x_ap[t * 128:(t + 1) * 128, :])
                    xsq = xw.tile([128, H], F32, tag="xsq", name=f"xsq{t}")
                    ssq = colp.tile([128, 1], F32, name=f"ssq{t}")
                    nc.scalar.activation(xsq[:], xt[:], Act.Square,
                                         accum_out=ssq[:])
                    sd = colp.tile([128, 1], F32, name=f"sd{t}")
                    nc.scalar.activation(sd[:], ssq[:], Act.Sqrt, bias=epsb[:],
                                         scale=1.0 / H)
                    rstd = colp.tile([128, 1], F32, name=f"rstd{t}")
                    nc.vector.reciprocal(rstd[:], sd[:])
                    # ACT applies the per-token rstd; DVE only does the
                    # g-mult and the amax reduce (keeps DVE queue short)
                    xr = xw.tile([128, H], F32, tag="xr", name=f"xr{t}")
                    nc.scalar.activation(xr[:], xt[:], Act.Copy,
                                         scale=rstd[:])
                    xn = xw.tile([128, H], F32, tag="xn", name=f"xn{t}")
                    nc.vector.tensor_tensor(xn[:], xr[:], g_bc[:], Alu.mult)
                    am = amax1[:, t:t + 1]
                    nc.vector.tensor_reduce(am, xn[:], axis=X, op=Alu.max,
                                            apply_absolute_value=True)
                    nc.vector.tensor_scalar_max(am, am, EPS)
                    rc = colp.tile([128, 1], F32, name=f"rc{t}")
                    nc.vector.reciprocal(rc[:], am)
                    s1 = colp.tile([128, 1], F32, name=f"s1{t}")
                    nc.vector.tensor_scalar_mul(s1[:], rc[:], 127.0)
                    # int8-grid round via the magic-constant trick on ACT
                    q1 = xw.tile([128, H], F32, tag="q1", name=f"q1{t}")
                    nc.scalar.activation(q1[:], xn[:], Act.Copy,
                                         scale=s1[:], bias=C_MAGIC)
                    q = xw.tile([128, H], BF16, tag="q", name=f"q{t}")
                    nc.scalar.activation(q[:], q1[:], Act.Copy,
                                         bias=-C_MAGIC)
                    for i in range(HK):
                        tps = psX.tile([128, 128], BF16, tag="tps",
                                       name=f"tps{t}_{i}")
                        nc.tensor.transpose(tps[:], q[:, i * 128:(i + 1) * 128],
                                            ident_b[:])
                        nc.scalar.copy(xqT[:, i * TPC + t * 128:
                                           i * TPC + (t + 1) * 128], tps[:])

            def cols_to_row_bcast(cols, name):
                with tc.tile_pool(name=f"psR{name}", bufs=1,
                                  space="PSUM") as psR:
                    ps = psR.tile([TT, 128], F32, tag="rowps",
                                  name=f"{name}_ps")
                    nc.tensor.transpose(ps[:], cols[:], ident_f[:])
                    r4 = colp.tile([TT, 128], F32, name=f"{name}_r4")
                    nc.scalar.copy(r4[:], ps[:])
                # bounce [4,128] -> [1,512] through DRAM (linear reinterp)
                rb = dram.tile([TT, 128], F32, name=f"{name}_rb")
                nc.sync.dma_start(rb[:], r4[:])
                row = colp.tile([1, TPC], F32, name=f"{name}_r")
                nc.sync.dma_start(row[:],
                                  rb[:].rearrange("a b -> (a b)").rearrange("(o f) -> o f", o=1))
                bc = colp.tile([128, TPC], F32, name=f"{name}_bc")
                nc.gpsimd.partition_broadcast(bc[:], row[:])
                return bc

            # ---- thresholds from the AllReduced stats ----
            def bcast_scaled(src, scale, name):
                t1 = colp.tile([1, 1], F32, name=f"{name}_s")
                nc.vector.tensor_scalar_mul(t1[:], src, scale)
                t2 = colp.tile([128, 1], F32, name=f"{name}_b")
                nc.gpsimd.partition_broadcast(t2[:], t1[:])
                return t2

            thr_g = bcast_scaled(stat_res[0:1, 0:1], 2.0 ** -26, "thrg")
            m_g = bcast_scaled(stat_res[0:1, 0:1], 2.0 ** -25, "mg")
            thr_d = bcast_scaled(stat_res[0:1, 1:2], 2.0 ** -25, "thrd")
            m_d = bcast_scaled(stat_res[0:1, 1:2], 2.0 ** -24, "md")
            nthr_g = colp.tile([128, 1], F32)
            nc.vector.tensor_scalar_mul(nthr_g[:], thr_g[:], -1.0)
            nthr_d = colp.tile([128, 1], F32)
            nc.vector.tensor_scalar_mul(nthr_d[:], thr_d[:], -1.0)

            ys_cols = colp.tile([128, TT], F32)
            nc.vector.tensor_scalar(ys_cols[:], amax1[:], m_g[:], 1.0 / 127.0,
                                    Alu.mult, Alu.mult)
            ys_bc = cols_to_row_bcast(ys_cols, "ys")

            # ---- P2: ternarize (fp8). chunks 0-1 now; 2-7 and the
            # down-proj emitted inside the mm1 loop. ----
            tg_shs = [dram.tile([GSH, 2 * GW], F8, name=f"tgsh{k}")
                      for k in range(NCH)]
            tg_fulls = [dram.tile([H, 2 * GW], F8, addr_space="Shared",
                                  name=f"tgf{k}") for k in range(NCH)]
            td_sh = dram.tile([DSH, H], F8)
            td_full = dram.tile([I, H], F8, addr_space="Shared")

            tp_ctx = tc.tile_pool(name="ternp", bufs=2)
            tp = tp_ctx.__enter__()

            def tern_cols(eng, w_ap, blk, src_c0, ncol, thr, nthr, dst,
                          dst_c0, nm):
                w = tp.tile([128, ncol], F32, tag="tw", name=f"tw{nm}")
                eng.dma_start(w[:], w_ap[blk * 128:(blk + 1) * 128,
                                         src_c0:src_c0 + ncol])
                a = tp.tile([128, ncol], BF16, tag="ta", name=f"ta{nm}")
                nc.vector.tensor_scalar(a[:], w[:], thr[:], 0.5,
                                        Alu.is_gt, Alu.subtract)
                b = tp.tile([128, ncol], BF16, tag="tb", name=f"tb{nm}")
                nc.vector.tensor_scalar(b[:], w[:], nthr[:], 0.5,
                                        Alu.is_ge, Alu.subtract)
                t = tp.tile([128, ncol], F8, tag="tc", name=f"tc{nm}")
                nc.vector.tensor_tensor(t[:], a[:], b[:], Alu.add)
                eng.dma_start(dst[blk * 128:(blk + 1) * 128,
                                  dst_c0:dst_c0 + ncol], t[:])

            def tern_gate_chunk(k, eng):
                for blk in range(GSH // 128):
                    tern_cols(eng, wg_ap, blk, k * GW, GW,
                              thr_g, nthr_g, tg_shs[k], 0, f"g{k}_{blk}")
                    tern_cols(eng, wg_ap, blk, I + k * GW, GW,
                              thr_g, nthr_g, tg_shs[k], GW, f"v{k}_{blk}")
                nc.gpsimd.collective_compute(
                    "AllGather", Alu.bypass, replica_groups=rg,
                    ins=[tg_shs[k][:]], outs=[tg_fulls[k][:]])

            tern_gate_chunk(0, nc.sync)
            tern_gate_chunk(1, nc.sync)

            # ---- P3: mm1 + SwiGLU -> h [I, tok] f32 spilled to DRAM,
            # with tern chunks 2-7 + down-proj tern emitted inside ----
            h_dram = dram.tile([I, TPC], F32)
            acc_sq = colp.tile([128, TPC], F32)
            nc.vector.memset(acc_sq[:], 0.0)
            acc_mxp = colp.tile([128, TPC], F32)
            nc.vector.memset(acc_mxp[:], -3.0e38)
            acc_mxn = colp.tile([128, TPC], F32)
            nc.vector.memset(acc_mxn[:], 3.0e38)
            tgvs = [t[:].rearrange("(i p) o -> p i o", p=128)
                    for t in tg_fulls]

            def tern_td_part(part):
                # 8 row-blocks split 2/2/2/2 across parts 0-3
                for blk in range(part * 2, part * 2 + 2):
                    tern_cols(nc.gpsimd, wd_ap, blk, 0, H, thr_d, nthr_d,
                              td_sh, 0, f"d{blk}")
                if part == 3:
                    nc.gpsimd.collective_compute(
                        "AllGather", Alu.bypass, replica_groups=rg,
                        ins=[td_sh[:]], outs=[td_full[:]])

            with tc.tile_pool(name="p3", bufs=3) as p3, \
                 tc.tile_pool(name="psMM1", bufs=2, space="PSUM") as psM1:
              for sb in range(16):  # o-blocks of 512 cols per half
                kg, og = sb // 2, (sb % 2) * 512
                tg_g = p3.tile([128, HK, 512], F8, tag="tg_g",
                               name=f"tgg{sb}")
                nc.sync.dma_start(tg_g[:], tgvs[kg][:, :, og:og + 512])
                tg_v = p3.tile([128, HK, 512], F8, tag="tg_v",
                               name=f"tgv{sb}")
                nc.sync.dma_start(tg_v[:],
                                  tgvs[kg][:, :, GW + og:GW + og + 512])
                for si in range(2):
                    s = sb * 2 + si
                    pg = [psM1.tile([128, TPC], F32, tag=f"pg{jj}",
                                    name=f"pg{s}_{jj}") for jj in range(2)]
                    pv = [psM1.tile([128, TPC], F32, tag=f"pv{jj}",
                                    name=f"pv{s}_{jj}") for jj in range(2)]
                    for i in range(HK):
                        rhs = xqT[:, i * TPC:(i + 1) * TPC]
                        st, sp = i == 0, i == HK - 1
                        for jj in range(2):
                            co = si * 256 + jj * 128
                            nc.tensor.matmul(
                                pg[jj][:], tg_g[:, i, co:co + 128],
                                rhs, start=st, stop=sp)
                            nc.tensor.matmul(
                                pv[jj][:], tg_v[:, i, co:co + 128],
                                rhs, start=st, stop=sp)
                    for jj in range(2):
                        # I-row block: chunk kg covers rows
                        # [kg*1024,(kg+1)*1024); offset og+si*256+jj*128
                        jb = kg * 8 + (sb % 2) * 4 + si * 2 + jj
                        gsc = p3.tile([128, TPC], F32, tag="gsc", name=f"gs{jb}")
                        nc.vector.tensor_tensor(gsc[:], pg[jj][:], ys_bc[:],
                                                Alu.mult)
                        sg = p3.tile([128, TPC], F32, tag="sg", name=f"sg{jb}")
                        nc.scalar.activation(sg[:], gsc[:], Act.Silu)
                        vsc = p3.tile([128, TPC], F32, tag="vsc", name=f"vs{jb}")
                        nc.vector.tensor_tensor(vsc[:], pv[jj][:], ys_bc[:],
                                                Alu.mult)
                        hj = p3.tile([128, TPC], F32, tag="hj", name=f"hj{jb}")
                        nc.vector.tensor_tensor(hj[:], sg[:], vsc[:], Alu.mult)
                        nc.sync.dma_start(h_dram[jb * 128:(jb + 1) * 128, :],
                                          hj[:])
                        hsq = p3.tile([128, TPC], F32, tag="hsq", name=f"hq{jb}")
                        nc.scalar.activation(hsq[:], hj[:], Act.Square)
                        nc.vector.tensor_tensor(acc_sq[:], acc_sq[:], hsq[:],
                                                Alu.add)
                        hg = p3.tile([128, TPC], F32, tag="hg", name=f"hg{jb}")
                        nc.vector.tensor_scalar_mul(hg[:], hj[:],
                                                    gdc_sb[:, jb:jb + 1])
                        nc.vector.tensor_tensor(acc_mxp[:], acc_mxp[:], hg[:],
                                                Alu.max)
                        nc.vector.tensor_tensor(acc_mxn[:], acc_mxn[:], hg[:],
                                                Alu.min)
                # interleaved tern emission: gate chunks 2-7 right after
                # sb 0-5 (their AGs must land well before consumption at
                # sb 4,6,..,14); down-proj tern after sb 6-9 so its AG
                # triggers last on the serial cc stream.
                if sb <= 5:
                    tern_gate_chunk(sb + 2, nc.gpsimd)
                if 6 <= sb <= 9:
                    tern_td_part(sb - 6)

            tp_ctx.__exit__(None, None, None)
            xq_p.__exit__(None, None, None)

            # ---- P4: per-token stats over I ----
            qs_cols = colp.tile([128, TT], F32)
            y2s_cols = colp.tile([128, TT], F32)
            ps4_ctx = tc.tile_pool(name="ps4", bufs=1, space="PSUM")
            ps4 = ps4_ctx.__enter__()
            for t in range(TT):
                sl = slice(t * 128, (t + 1) * 128)
                pssq = ps4.tile([128, 128], F32, tag="pssq", name=f"pq{t}")
                nc.tensor.transpose(pssq[:], acc_sq[:, sl], ident_f[:])
                ss = colp.tile([128, 1], F32, name=f"hss{t}")
                nc.vector.tensor_reduce(ss[:], pssq[:], axis=X, op=Alu.add)
                psm1 = ps4.tile([128, 128], F32, tag="psm1", name=f"pm1{t}")
                nc.tensor.transpose(psm1[:], acc_mxp[:, sl], ident_f[:])
                c1 = colp.tile([128, 1], F32, name=f"hc1{t}")
                nc.vector.tensor_reduce(c1[:], psm1[:], axis=X, op=Alu.max,
                                        apply_absolute_value=True)
                psm2 = ps4.tile([128, 128], F32, tag="psm2", name=f"pm2{t}")
                nc.tensor.transpose(psm2[:], acc_mxn[:, sl], ident_f[:])
                c2 = colp.tile([128, 1], F32, name=f"hc2{t}")
                nc.vector.tensor_reduce(c2[:], psm2[:], axis=X, op=Alu.max,
                                        apply_absolute_value=True)
                amax_hg = colp.tile([128, 1], F32, name=f"amhg{t}")
                nc.vector.tensor_tensor(amax_hg[:], c1[:], c2[:], Alu.max)
                sd2 = colp.tile([128, 1], F32, name=f"sd2{t}")
                nc.scalar.activation(sd2[:], ss[:], Act.Sqrt, bias=epsb[:],
                                     scale=1.0 / I)
                rstd2 = colp.tile([128, 1], F32, name=f"rstd2{t}")
                nc.vector.reciprocal(rstd2[:], sd2[:])
                t1 = colp.tile([128, 1], F32, name=f"t1{t}")
                nc.vector.tensor_scalar(t1[:], amax_hg[:], rstd2[:], EPS,
                                        Alu.mult, Alu.max)
                rc2 = colp.tile([128, 1], F32, name=f"rc2{t}")
                nc.vector.reciprocal(rc2[:], t1[:])
                s2 = colp.tile([128, 1], F32, name=f"s2{t}")
                nc.vector.tensor_scalar_mul(s2[:], rc2[:], 127.0)
                nc.vector.tensor_scalar_mul(qs_cols[:, t:t + 1], rstd2[:],
                                            s2[:])
                nc.vector.tensor_scalar(y2s_cols[:, t:t + 1], t1[:], m_d[:],
                                        1.0 / 127.0, Alu.mult, Alu.mult)
            ps4_ctx.__exit__(None, None, None)
            qs_bc = cols_to_row_bcast(qs_cols, "qs")

            # ---- P5: j-outer mm2, 2 H-halves; quant fused in half 0,
            # q2T cached in SBUF (bf16) for half 1 ----
            tdv = td_full[:].rearrange("(j p) o -> p j o", p=128)
            q2p_ctx = tc.tile_pool(name="q2p", bufs=1)
            q2p = q2p_ctx.__enter__()
            q2T = q2p.tile([128, IK * TPC], BF16)
            with tc.tile_pool(name="p5", bufs=3) as p5, \
                 tc.tile_pool(name="psMM2", bufs=1, space="PSUM") as psM2:
                for half in range(2):
                    p2 = [psM2.tile([128, 512], F32, tag=f"p2_{q}",
                                    name=f"p2_{half}_{q}") for q in range(8)]
                    for j in range(IK):
                        if half == 0:
                            hjl = p5.tile([128, TPC], F32, tag="hjl",
                                          name=f"h4_{j}")
                            nc.sync.dma_start(
                                hjl[:], h_dram[j * 128:(j + 1) * 128, :])
                            hg2 = p5.tile([128, TPC], F32, tag="hg2",
                                          name=f"g4_{j}")
                            nc.vector.tensor_scalar_mul(hg2[:], hjl[:],
                                                        gdc_sb[:, j:j + 1])
                            nc.vector.tensor_tensor(hg2[:], hg2[:], qs_bc[:],
                                                    Alu.mult)
                            nc.vector.tensor_scalar(
                                q2T[:, j * TPC:(j + 1) * TPC], hg2[:],
                                C_MAGIC, C_MAGIC, Alu.add, Alu.subtract)
                        td_j = p5.tile([128, 1024], F8, tag="td",
                                       name=f"td{half}_{j}")
                        nc.scalar.dma_start(
                            td_j[:], tdv[:, j, half * 1024:(half + 1) * 1024])
                        st, sp = j == 0, j == IK - 1
                        for t in range(TT):
                            for hq in range(2):
                                nc.tensor.matmul(
                                    p2[t * 2 + hq][:],
                                    q2T[:, j * TPC + t * 128:
                                        j * TPC + (t + 1) * 128],
                                    td_j[:, hq * 512:(hq + 1) * 512],
                                    start=st, stop=sp)
                    for t in range(TT):
                        for hq in range(2):
                            yt = p5.tile([128, 512], F32, tag="yt",
                                         name=f"yt{half}_{t}_{hq}")
                            nc.vector.tensor_scalar_mul(
                                yt[:], p2[t * 2 + hq][:],
                                y2s_cols[:, t:t + 1])
                            nc.sync.dma_start(
                                y_ap[t * 128:(t + 1) * 128,
                                     half * 1024 + hq * 512:
                                     half * 1024 + (hq + 1) * 512], yt[:])
            q2p_ctx.__exit__(None, None, None)
    return nc


_CACHE = {}


def _get_compiled():
    if "nc" not in _CACHE:
        nc = bacc.Bacc("TRN2", target_bir_lowering=False, debug=False,
                       enable_asserts=False, num_devices=NC_N)
        build(nc)
        nc.compile()
        _CACHE["nc"] = nc
    return _CACHE["nc"]


def kernel(x, w_gate, g_gate, w_down, g_down):
    nc = _get_compiled()
    x2 = np.ascontiguousarray(np.asarray(x, np.float32).reshape(TOK, H))
    wgT = np.asarray(w_gate, np.float32).T
    wdT = np.asarray(w_down, np.float32).T
    gg = np.ascontiguousarray(np.asarray(g_gate, np.float32).reshape(1, H))
    gdc = np.ascontiguousarray(
        np.asarray(g_down, np.float32).reshape(IK, 128).T)
    in_maps = []
    for c in range(NC_N):
        in_maps.append({
            "x": x2[c * TPC:(c + 1) * TPC],
            "wgt": np.ascontiguousarray(wgT[c * GSH:(c + 1) * GSH]),
            "wdt": np.ascontiguousarray(wdT[c * DSH:(c + 1) * DSH]),
            "gg": gg,
            "gdc": gdc,
        })
    res = run_bass_kernel_spmd(nc, in_maps, core_ids=list(range(NC_N)))
    out = np.concatenate([res.results[c]["y"] for c in range(NC_N)], axis=0)
    return out.reshape(B, S, H).astype(np.float32)


# revision 37
# speedup vs baseline: 1.1179x; 1.0207x over previous
"""HGRNBitMLP (BitNet-style SwiGLU MLP) on 8 TRN2 NeuronCores.

Data-parallel over the 4096 tokens (512/core). Weight ternarization is
sharded 1/8 per core; ternary weights are stored as fp8e4 (exact for
{-1,0,+1}) and AllGathered in fp8 — half the bytes of bf16. The global
mean(|w|) comes from two tiny AllReduces. Activations are quantized to
the int8 grid (exact in bf16), so every matmul is an exact-integer
fp8xbf16 matmul with f32 PSUM accumulation. Per-token scales are
applied outside the matmuls.

Engine-queue discipline (strict FIFO per engine):
 - program order: inits -> x-quant (P0x) -> weight stats/AR -> tern
   chunks 0-1 -> mm1 loop with tern chunks 2-7 and the down-proj tern
   EMITTED INSIDE the loop (so their DVE ops interleave with SwiGLU
   instead of damming the queue).
 - x + gate-tile loads ride the scalar(ACT) / sync HWDGE rings; the
   interleaved tern DMA traffic rides the gpsimd SWDGE ring, keeping
   the sync ring free for mm1's tile loads and h spills.
 - h-square on ACT, stats accumulators on DVE: the gpsimd queue stays
   clear for collective triggers.

Layouts: x is loaded [tok, h], quantized, PE-transposed to xqT [h, tok].
mm1 produces y^T tiles [o, tok]; SwiGLU keeps h as [I, tok] (f32,
spilled to DRAM) so mm2's operand q2T [I, tok] needs no transpose.
mm2 is j-outer over I with quantization fused into the first H-half
(q2T cached in SBUF bf16 for the second half) and computes out
[tok, H] in two H-halves.
"""
import sys

try:
    import concourse  # noqa: F401
except ImportError:
    sys.path.insert(0, "/opt/trn_rl_repo")

import numpy as np

import concourse.tile as tile
from concourse import bacc, mybir
from concourse.bass_utils import run_bass_kernel_spmd
from concourse.masks import make_identity

F32, BF16, F8 = mybir.dt.float32, mybir.dt.bfloat16, mybir.dt.float8e4
Alu = mybir.AluOpType
Act = mybir.ActivationFunctionType
X = mybir.AxisListType.X

NC_N = 8
B, S, H, I = 2, 2048, 2048, 8192
O2 = 2 * I
TOK = B * S
TPC = TOK // NC_N   # 512 tokens/core
TT = TPC // 128     # 4 token tiles
HK = H // 128       # 16 h tiles
IK = I // 128       # 64 I tiles
GSH = H // NC_N     # 256 rows of w_gate^T per core
DSH = I // NC_N     # 1024 rows of w_down^T per core
EPS = 1e-5
C_MAGIC = 12582912.0  # 1.5*2^23; (x+C)-C rounds f32 to nearest-even int
CCHUNK = 2048
NCH = 8             # gate AG chunks; each pairs 1024 gate + 1024 v cols
GW = O2 // NCH // 2  # 1024 och of gate (and of v) per chunk


def build(nc):
    x_ap = nc.dram_tensor("x", [TPC, H], F32, kind="ExternalInput").ap()
    wg_ap = nc.dram_tensor("wgt", [GSH, O2], F32, kind="ExternalInput").ap()
    wd_ap = nc.dram_tensor("wdt", [DSH, H], F32, kind="ExternalInput").ap()
    gg_ap = nc.dram_tensor("gg", [1, H], F32, kind="ExternalInput").ap()
    gd_ap = nc.dram_tensor("gdc", [128, IK], F32, kind="ExternalInput").ap()
    y_ap = nc.dram_tensor("y", [TPC, H], F32, kind="ExternalOutput").ap()
    rg = [list(range(NC_N))]

    with tile.TileContext(nc) as tc:
        with tc.tile_pool(name="dram", bufs=1, space="DRAM") as dram, \
             tc.tile_pool(name="perm", bufs=1) as cp, \
             tc.tile_pool(name="colp", bufs=1) as colp:

            # ---- inits (before any collective hits the gpsimd queue) --
            ones = cp.tile([128, 1], F32)
            nc.gpsimd.memset(ones[:], 1.0)
            epsb = cp.tile([128, 1], F32)
            nc.gpsimd.memset(epsb[:], EPS)
            ident_b = cp.tile([128, 128], BF16)
            make_identity(nc, ident_b[:])
            ident_f = cp.tile([128, 128], F32)
            make_identity(nc, ident_f[:])
            gdc_sb = cp.tile([128, IK], F32)
            nc.scalar.dma_start(gdc_sb[:], gd_ap[:])

            # warmup collective: the first collective after the init
            # barrier pays ~20us extra; burn that on a dummy.
            warm_sb = cp.tile([1, 1], F32)
            nc.vector.memset(warm_sb[:], 0.0)
            warm_in = dram.tile([1, 1], F32, name="warm_in")
            warm_out = dram.tile([8, 1], F32, addr_space="Shared",
                                 name="warm_out")
            nc.sync.dma_start(warm_in[:], warm_sb[:])
            nc.gpsimd.collective_compute("AllGather", Alu.bypass,
                                         replica_groups=rg,
                                         ins=[warm_in[:]], outs=[warm_out[:]])

            # ---- P0w: sharded abs-sums; ONE tiny AllGather + on-PE
            # partition sum. First on the DVE FIFO: its reduces are
            # read-paced, so they finish by ~90us and the stats AG
            # fires ~70us earlier than when queued behind P0x. ----
            stat_sb = colp.tile([1, 2], F32)
            with tc.tile_pool(name="statp", bufs=3) as stp, \
                 tc.tile_pool(name="psStat", bufs=1, space="PSUM") as psS:
                def stat_pass(w_ap, nblk, ncol, key, col):
                    parts = colp.tile([128, nblk * ncol], F32, name=f"pt{key}")
                    for blk in range(nblk):
                        for ck in range(ncol):
                            wch = stp.tile([128, CCHUNK], F32, tag="wch",
                                           name=f"w{key}{blk}_{ck}")
                            nc.sync.dma_start(
                                wch[:], w_ap[blk * 128:(blk + 1) * 128,
                                             ck * CCHUNK:(ck + 1) * CCHUNK])
                            nc.vector.tensor_reduce(
                                parts[:, blk * ncol + ck:blk * ncol + ck + 1],
                                wch[:], axis=X, op=Alu.add,
                                apply_absolute_value=True)
                    sums = colp.tile([128, 1], F32, name=f"sm{key}")
                    nc.vector.tensor_reduce(sums[:], parts[:], axis=X,
                                            op=Alu.add)
                    ps = psS.tile([1, 1], F32, tag=f"sps{key}",
                                  name=f"sps{key}")
                    nc.tensor.matmul(ps[:], sums[:], ones[:], start=True,
                                     stop=True)
                    nc.scalar.copy(stat_sb[:, col:col + 1], ps[:])

                stat_pass(wg_ap, 2, O2 // CCHUNK, "g", 0)
                stat_pass(wd_ap, 8, 1, "d", 1)

                sin = dram.tile([1, 2], F32, name="statin")
                sout = dram.tile([8, 2], F32, addr_space="Shared",
                                 name="statout")
                nc.sync.dma_start(sin[:], stat_sb[:])
                nc.gpsimd.collective_compute(
                    "AllGather", Alu.bypass, replica_groups=rg,
                    ins=[sin[:]], outs=[sout[:]])
                res8 = colp.tile([8, 2], F32, name="res8")
                nc.sync.dma_start(res8[:], sout[:])
                ps2 = psS.tile([1, 2], F32, tag="spsum", name="spsum")
                nc.tensor.matmul(ps2[:], ones[0:8, 0:1], res8[:],
                                 start=True, stop=True)
                stat_res = colp.tile([1, 2], F32, name="statres")
                nc.scalar.copy(stat_res[:], ps2[:])

            # ---- thresholds from the AllGathered stats ----
            def bcast_scaled(src, scale, name):
                t1 = colp.tile([1, 1], F32, name=f"{name}_s")
                nc.vector.tensor_scalar_mul(t1[:], src, scale)
                t2 = colp.tile([128, 1], F32, name=f"{name}_b")
                nc.gpsimd.partition_broadcast(t2[:], t1[:])
                return t2

            thr_g = bcast_scaled(stat_res[0:1, 0:1], 2.0 ** -26, "thrg")
            m_g = bcast_scaled(stat_res[0:1, 0:1], 2.0 ** -25, "mg")
            thr_d = bcast_scaled(stat_res[0:1, 1:2], 2.0 ** -25, "thrd")
            m_d = bcast_scaled(stat_res[0:1, 1:2], 2.0 ** -24, "md")
            nthr_g = colp.tile([128, 1], F32)
            nc.vector.tensor_scalar_mul(nthr_g[:], thr_g[:], -1.0)
            nthr_d = colp.tile([128, 1], F32)
            nc.vector.tensor_scalar_mul(nthr_d[:], thr_d[:], -1.0)

            # ---- P2: ternarize (fp8). chunks 0-1 BEFORE x-quant (their
            # DVE ops only wait on the stats AG, so AG chunk 0 fires
            # ~40us earlier; P0x's DVE isn't needed until its PE
            # transposes, well after). chunks 2-7 and the down-proj are
            # emitted inside the mm1 loop. ----
            tg_shs = [dram.tile([GSH, 2 * GW], F8, name=f"tgsh{k}")
                      for k in range(NCH)]
            tg_fulls = [dram.tile([H, 2 * GW], F8, addr_space="Shared",
                                  name=f"tgf{k}") for k in range(NCH)]
            td_sh = dram.tile([DSH, H], F8)
            td_full = dram.tile([I, H], F8, addr_space="Shared")

            tp_ctx = tc.tile_pool(name="ternp", bufs=2)
            tp = tp_ctx.__enter__()

            def tern_cols(eng, w_ap, blk, src_c0, ncol, thr, nthr, dst,
                          dst_c0, nm):
                w = tp.tile([128, ncol], F32, tag="tw", name=f"tw{nm}")
                eng.dma_start(w[:], w_ap[blk * 128:(blk + 1) * 128,
                                         src_c0:src_c0 + ncol])
                a = tp.tile([128, ncol], BF16, tag="ta", name=f"ta{nm}")
                nc.vector.tensor_scalar(a[:], w[:], thr[:], 0.5,
                                        Alu.is_gt, Alu.subtract)
                b = tp.tile([128, ncol], BF16, tag="tb", name=f"tb{nm}")
                nc.vector.tensor_scalar(b[:], w[:], nthr[:], 0.5,
                                        Alu.is_ge, Alu.subtract)
                t = tp.tile([128, ncol], F8, tag="tc", name=f"tc{nm}")
                nc.vector.tensor_tensor(t[:], a[:], b[:], Alu.add)
                eng.dma_start(dst[blk * 128:(blk + 1) * 128,
                                  dst_c0:dst_c0 + ncol], t[:])

            def tern_gate_chunk(k, eng):
                for blk in range(GSH // 128):
                    tern_cols(eng, wg_ap, blk, k * GW, GW,
                              thr_g, nthr_g, tg_shs[k], 0, f"g{k}_{blk}")
                    tern_cols(eng, wg_ap, blk, I + k * GW, GW,
                              thr_g, nthr_g, tg_shs[k], GW, f"v{k}_{blk}")
                nc.gpsimd.collective_compute(
                    "AllGather", Alu.bypass, replica_groups=rg,
                    ins=[tg_shs[k][:]], outs=[tg_fulls[k][:]])

            tern_gate_chunk(0, nc.sync)
            tern_gate_chunk(1, nc.sync)

            # ---- P0x: x rmsnorm + int8-grid quant + transpose ----
            xq_p = tc.tile_pool(name="xqp", bufs=1)
            xq_pool = xq_p.__enter__()
            gg_sb = xq_pool.tile([1, H], F32)
            nc.scalar.dma_start(gg_sb[:], gg_ap[:])
            g_bc = xq_pool.tile([128, H], F32)
            nc.gpsimd.partition_broadcast(g_bc[:], gg_sb[:])
            xqT = xq_pool.tile([128, HK * TPC], BF16)
            amax1 = colp.tile([128, TT], F32)
            with tc.tile_pool(name="xwork", bufs=2) as xw, \
                 tc.tile_pool(name="psX", bufs=2, space="PSUM") as psX:
                for t in range(TT):
                    xt = xw.tile([128, H], F32, tag="xt", name=f"xt{t}")
                    nc.scalar.dma_start(xt[:], x_ap[t * 128:(t + 1) * 128, :])
                    xsq = xw.tile([128, H], F32, tag="xsq", name=f"xsq{t}")
                    ssq = colp.tile([128, 1], F32, name=f"ssq{t}")
                    nc.scalar.activation(xsq[:], xt[:], Act.Square,
                                         accum_out=ssq[:])
                    sd = colp.tile([128, 1], F32, name=f"sd{t}")
                    nc.scalar.activation(sd[:], ssq[:], Act.Sqrt, bias=epsb[:],
                                         scale=1.0 / H)
                    rstd = colp.tile([128, 1], F32, name=f"rstd{t}")
                    nc.vector.reciprocal(rstd[:], sd[:])
                    # ACT applies the per-token rstd; DVE only does the
                    # g-mult and the amax reduce (keeps DVE queue short)
                    xr = xw.tile([128, H], F32, tag="xr", name=f"xr{t}")
                    nc.scalar.activation(xr[:], xt[:], Act.Copy,
                                         scale=rstd[:])
                    xn = xw.tile([128, H], F32, tag="xn", name=f"xn{t}")
                    nc.vector.tensor_tensor(xn[:], xr[:], g_bc[:], Alu.mult)
                    am = amax1[:, t:t + 1]
                    nc.vector.tensor_reduce(am, xn[:], axis=X, op=Alu.max,
                                            apply_absolute_value=True)
                    nc.vector.tensor_scalar_max(am, am, EPS)
                    rc = colp.tile([128, 1], F32, name=f"rc{t}")
                    nc.vector.reciprocal(rc[:], am)
                    s1 = colp.tile([128, 1], F32, name=f"s1{t}")
                    nc.vector.tensor_scalar_mul(s1[:], rc[:], 127.0)
                    # int8-grid round via the magic-constant trick on ACT
                    q1 = xw.tile([128, H], F32, tag="q1", name=f"q1{t}")
                    nc.scalar.activation(q1[:], xn[:], Act.Copy,
                                         scale=s1[:], bias=C_MAGIC)
                    q = xw.tile([128, H], BF16, tag="q", name=f"q{t}")
                    nc.scalar.activation(q[:], q1[:], Act.Copy,
                                         bias=-C_MAGIC)
                    for i in range(HK):
                        tps = psX.tile([128, 128], BF16, tag="tps",
                                       name=f"tps{t}_{i}")
                        nc.tensor.transpose(tps[:], q[:, i * 128:(i + 1) * 128],
                                            ident_b[:])
                        nc.scalar.copy(xqT[:, i * TPC + t * 128:
                                           i * TPC + (t + 1) * 128], tps[:])

            def cols_to_row_bcast(cols, name):
                with tc.tile_pool(name=f"psR{name}", bufs=1,
                                  space="PSUM") as psR:
                    ps = psR.tile([TT, 128], F32, tag="rowps",
                                  name=f"{name}_ps")
                    nc.tensor.transpose(ps[:], cols[:], ident_f[:])
                    r4 = colp.tile([TT, 128], F32, name=f"{name}_r4")
                    nc.scalar.copy(r4[:], ps[:])
                # bounce [4,128] -> [1,512] through DRAM (linear reinterp)
                rb = dram.tile([TT, 128], F32, name=f"{name}_rb")
                nc.sync.dma_start(rb[:], r4[:])
                row = colp.tile([1, TPC], F32, name=f"{name}_r")
                nc.sync.dma_start(row[:],
                                  rb[:].rearrange("a b -> (a b)").rearrange("(o f) -> o f", o=1))
                bc = colp.tile([128, TPC], F32, name=f"{name}_bc")
                nc.gpsimd.partition_broadcast(bc[:], row[:])
                return bc

            ys_cols = colp.tile([128, TT], F32)
            nc.vector.tensor_scalar(ys_cols[:], amax1[:], m_g[:], 1.0 / 127.0,
                                    Alu.mult, Alu.mult)
            ys_bc = cols_to_row_bcast(ys_cols, "ys")

            # ---- P3: mm1 + SwiGLU -> h [I, tok] f32 spilled to DRAM,
            # with tern chunks 2-7 + down-proj tern emitted inside ----
            h_dram = dram.tile([I, TPC], F32)
            acc_sq = colp.tile([128, TPC], F32)
            nc.vector.memset(acc_sq[:], 0.0)
            acc_mxp = colp.tile([128, TPC], F32)
            nc.vector.memset(acc_mxp[:], -3.0e38)
            acc_mxn = colp.tile([128, TPC], F32)
            nc.vector.memset(acc_mxn[:], 3.0e38)
            tgvs = [t[:].rearrange("(i p) o -> p i o", p=128)
                    for t in tg_fulls]

            def tern_td_part(part):
                # 8 row-blocks split 2/2/2/2 across parts 0-3
                for blk in range(part * 2, part * 2 + 2):
                    tern_cols(nc.gpsimd, wd_ap, blk, 0, H, thr_d, nthr_d,
                              td_sh, 0, f"d{blk}")
                if part == 3:
                    nc.gpsimd.collective_compute(
                        "AllGather", Alu.bypass, replica_groups=rg,
                        ins=[td_sh[:]], outs=[td_full[:]])

            with tc.tile_pool(name="p3", bufs=3) as p3, \
                 tc.tile_pool(name="psMM1", bufs=2, space="PSUM") as psM1:
              for sb in range(16):  # o-blocks of 512 cols per half
                kg, og = sb // 2, (sb % 2) * 512
                tg_g = p3.tile([128, HK, 512], F8, tag="tg_g",
                               name=f"tgg{sb}")
                nc.sync.dma_start(tg_g[:], tgvs[kg][:, :, og:og + 512])
                tg_v = p3.tile([128, HK, 512], F8, tag="tg_v",
                               name=f"tgv{sb}")
                nc.sync.dma_start(tg_v[:],
                                  tgvs[kg][:, :, GW + og:GW + og + 512])
                for si in range(2):
                    s = sb * 2 + si
                    pg = [psM1.tile([128, TPC], F32, tag=f"pg{jj}",
                                    name=f"pg{s}_{jj}") for jj in range(2)]
                    pv = [psM1.tile([128, TPC], F32, tag=f"pv{jj}",
                                    name=f"pv{s}_{jj}") for jj in range(2)]
                    for i in range(HK):
                        rhs = xqT[:, i * TPC:(i + 1) * TPC]
                        st, sp = i == 0, i == HK - 1
                        for jj in range(2):
                            co = si * 256 + jj * 128
                            nc.tensor.matmul(
                                pg[jj][:], tg_g[:, i, co:co + 128],
                                rhs, start=st, stop=sp)
                            nc.tensor.matmul(
                                pv[jj][:], tg_v[:, i, co:co + 128],
                                rhs, start=st, stop=sp)
                    for jj in range(2):
                        # I-row block: chunk kg covers rows
                        # [kg*1024,(kg+1)*1024); offset og+si*256+jj*128
                        jb = kg * 8 + (sb % 2) * 4 + si * 2 + jj
                        gsc = p3.tile([128, TPC], F32, tag="gsc", name=f"gs{jb}")
                        nc.vector.tensor_tensor(gsc[:], pg[jj][:], ys_bc[:],
                                                Alu.mult)
                        sg = p3.tile([128, TPC], F32, tag="sg", name=f"sg{jb}")
                        nc.scalar.activation(sg[:], gsc[:], Act.Silu)
                        vsc = p3.tile([128, TPC], F32, tag="vsc", name=f"vs{jb}")
                        nc.vector.tensor_tensor(vsc[:], pv[jj][:], ys_bc[:],
                                                Alu.mult)
                        hj = p3.tile([128, TPC], F32, tag="hj", name=f"hj{jb}")
                        nc.vector.tensor_tensor(hj[:], sg[:], vsc[:], Alu.mult)
                        nc.sync.dma_start(h_dram[jb * 128:(jb + 1) * 128, :],
                                          hj[:])
                        hsq = p3.tile([128, TPC], F32, tag="hsq", name=f"hq{jb}")
                        nc.scalar.activation(hsq[:], hj[:], Act.Square)
                        nc.vector.tensor_tensor(acc_sq[:], acc_sq[:], hsq[:],
                                                Alu.add)
                        hg = p3.tile([128, TPC], F32, tag="hg", name=f"hg{jb}")
                        nc.vector.tensor_scalar_mul(hg[:], hj[:],
                                                    gdc_sb[:, jb:jb + 1])
                        nc.vector.tensor_tensor(acc_mxp[:], acc_mxp[:], hg[:],
                                                Alu.max)
                        nc.vector.tensor_tensor(acc_mxn[:], acc_mxn[:], hg[:],
                                                Alu.min)
                # interleaved tern emission: gate chunks 2-7 right after
                # sb 0-5 (their AGs must land well before consumption at
                # sb 4,6,..,14); down-proj tern after sb 6-9 so its AG
                # triggers last on the serial cc stream.
                if sb <= 5:
                    tern_gate_chunk(sb + 2, nc.gpsimd)
                if 6 <= sb <= 9:
                    tern_td_part(sb - 6)

            xq_p.__exit__(None, None, None)
            tp_ctx.__exit__(None, None, None)

            # ---- P4: per-token stats over I ----
            qs_cols = colp.tile([128, TT], F32)
            y2s_cols = colp.tile([128, TT], F32)
            ps4_ctx = tc.tile_pool(name="ps4", bufs=1, space="PSUM")
            ps4 = ps4_ctx.__enter__()
            for t in range(TT):
                sl = slice(t * 128, (t + 1) * 128)
                pssq = ps4.tile([128, 128], F32, tag="pssq", name=f"pq{t}")
                nc.tensor.transpose(pssq[:], acc_sq[:, sl], ident_f[:])
                ss = colp.tile([128, 1], F32, name=f"hss{t}")
                nc.vector.tensor_reduce(ss[:], pssq[:], axis=X, op=Alu.add)
                psm1 = ps4.tile([128, 128], F32, tag="psm1", name=f"pm1{t}")
                nc.tensor.transpose(psm1[:], acc_mxp[:, sl], ident_f[:])
                c1 = colp.tile([128, 1], F32, name=f"hc1{t}")
                nc.vector.tensor_reduce(c1[:], psm1[:], axis=X, op=Alu.max,
                                        apply_absolute_value=True)
                psm2 = ps4.tile([128, 128], F32, tag="psm2", name=f"pm2{t}")
                nc.tensor.transpose(psm2[:], acc_mxn[:, sl], ident_f[:])
                c2 = colp.tile([128, 1], F32, name=f"hc2{t}")
                nc.vector.tensor_reduce(c2[:], psm2[:], axis=X, op=Alu.max,
                                        apply_absolute_value=True)
                amax_hg = colp.tile([128, 1], F32, name=f"amhg{t}")
                nc.vector.tensor_tensor(amax_hg[:], c1[:], c2[:], Alu.max)
                sd2 = colp.tile([128, 1], F32, name=f"sd2{t}")
                nc.scalar.activation(sd2[:], ss[:], Act.Sqrt, bias=epsb[:],
                                     scale=1.0 / I)
                rstd2 = colp.tile([128, 1], F32, name=f"rstd2{t}")
                nc.vector.reciprocal(rstd2[:], sd2[:])
                t1 = colp.tile([128, 1], F32, name=f"t1{t}")
                nc.vector.tensor_scalar(t1[:], amax_hg[:], rstd2[:], EPS,
                                        Alu.mult, Alu.max)
                rc2 = colp.tile([128, 1], F32, name=f"rc2{t}")
                nc.vector.reciprocal(rc2[:], t1[:])
                s2 = colp.tile([128, 1], F32, name=f"s2{t}")
                nc.vector.tensor_scalar_mul(s2[:], rc2[:], 127.0)
                nc.vector.tensor_scalar_mul(qs_cols[:, t:t + 1], rstd2[:],
                                            s2[:])
                nc.vector.tensor_scalar(y2s_cols[:, t:t + 1], t1[:], m_d[:],
                                        1.0 / 127.0, Alu.mult, Alu.mult)
            ps4_ctx.__exit__(None, None, None)
            qs_bc = cols_to_row_bcast(qs_cols, "qs")

            # ---- P5: j-outer mm2, 2 H-halves; quant fused in half 0,
            # q2T cached in SBUF (bf16) for half 1 ----
            tdv = td_full[:].rearrange("(j p) o -> p j o", p=128)
            q2p_ctx = tc.tile_pool(name="q2p", bufs=1)
            q2p = q2p_ctx.__enter__()
            q2T = q2p.tile([128, IK * TPC], BF16)
            with tc.tile_pool(name="p5", bufs=3) as p5, \
                 tc.tile_pool(name="psMM2", bufs=1, space="PSUM") as psM2:
                for half in range(2):
                    p2 = [psM2.tile([128, 512], F32, tag=f"p2_{q}",
                                    name=f"p2_{half}_{q}") for q in range(8)]
                    for j in range(IK):
                        if half == 0:
                            hjl = p5.tile([128, TPC], F32, tag="hjl",
                                          name=f"h4_{j}")
                            nc.sync.dma_start(
                                hjl[:], h_dram[j * 128:(j + 1) * 128, :])
                            hg2 = p5.tile([128, TPC], F32, tag="hg2",
                                          name=f"g4_{j}")
                            nc.vector.tensor_scalar_mul(hg2[:], hjl[:],
                                                        gdc_sb[:, j:j + 1])
                            nc.vector.tensor_tensor(hg2[:], hg2[:], qs_bc[:],
                                                    Alu.mult)
                            nc.vector.tensor_scalar(
                                q2T[:, j * TPC:(j + 1) * TPC], hg2[:],
                                C_MAGIC, C_MAGIC, Alu.add, Alu.subtract)
                        td_j = p5.tile([128, 1024], F8, tag="td",
                                       name=f"td{half}_{j}")
                        nc.scalar.dma_start(
                            td_j[:], tdv[:, j, half * 1024:(half + 1) * 1024])
                        st, sp = j == 0, j == IK - 1
                        for t in range(TT):
                            for hq in range(2):
                                nc.tensor.matmul(
                                    p2[t * 2 + hq][:],
                                    q2T[:, j * TPC + t * 128:
                                        j * TPC + (t + 1) * 128],
                                    td_j[:, hq * 512:(hq + 1) * 512],
                                    start=st, stop=sp)
                    for t in range(TT):
                        for hq in range(2):
                            yt = p5.tile([128, 512], F32, tag="yt",
                                         name=f"yt{half}_{t}_{hq}")
                            nc.vector.tensor_scalar_mul(
                                yt[:], p2[t * 2 + hq][:],
                                y2s_cols[:, t:t + 1])
                            nc.sync.dma_start(
                                y_ap[t * 128:(t + 1) * 128,
                                     half * 1024 + hq * 512:
                                     half * 1024 + (hq + 1) * 512], yt[:])
            q2p_ctx.__exit__(None, None, None)
    return nc


_CACHE = {}


def _get_compiled():
    if "nc" not in _CACHE:
        nc = bacc.Bacc("TRN2", target_bir_lowering=False, debug=False,
                       enable_asserts=False, num_devices=NC_N)
        build(nc)
        nc.compile()
        _CACHE["nc"] = nc
    return _CACHE["nc"]


def kernel(x, w_gate, g_gate, w_down, g_down):
    nc = _get_compiled()
    x2 = np.ascontiguousarray(np.asarray(x, np.float32).reshape(TOK, H))
    wgT = np.asarray(w_gate, np.float32).T
    wdT = np.asarray(w_down, np.float32).T
    gg = np.ascontiguousarray(np.asarray(g_gate, np.float32).reshape(1, H))
    gdc = np.ascontiguousarray(
        np.asarray(g_down, np.float32).reshape(IK, 128).T)
    in_maps = []
    for c in range(NC_N):
        in_maps.append({
            "x": x2[c * TPC:(c + 1) * TPC],
            "wgt": np.ascontiguousarray(wgT[c * GSH:(c + 1) * GSH]),
            "wdt": np.ascontiguousarray(wdT[c * DSH:(c + 1) * DSH]),
            "gg": gg,
            "gdc": gdc,
        })
    res = run_bass_kernel_spmd(nc, in_maps, core_ids=list(range(NC_N)))
    out = np.concatenate([res.results[c]["y"] for c in range(NC_N)], axis=0)
    return out.reshape(B, S, H).astype(np.float32)
